# revision 1
# baseline (speedup 1.0000x reference)
"""Trainium2 Bass kernel for nn_MemoryNetwork (GRU-style memory network scan).

Model (per reference):
  t_enc = cos(arange(T) * freq + phase)                    [T, D]
  s0 = mean_t(x)                                           [B*C, D]
  tr = arange(T) * mask; x_seq = x[tr]; te_seq = t_enc[tr]
  per step t:
    msg = gelu([x_t, s, te_t] @ msg_W.T + msg_b)
    gi = msg @ W_ih.T + b_ih ; gh = s @ W_hh.T + b_hh
    r = sigmoid(i_r + h_r); z = sigmoid(i_z + h_z)
    n = tanh(i_n + r * h_n)
    s' = (1 - z) * n + z * s
  output: states [T, B, C, D]

Strategy: data-parallel over B*C = 4096 rows -> 8 cores x 512 rows.
On-device layout is feature-major ([D, rows]); matmuls contract over the
partition dim. The 512 rows per core are split into NB=4 independent
blocks whose per-step dependency chains interleave, shrinking the serial
chain's per-op durations (the scan is latency-bound, not
throughput-bound).

One ACT table set (gelu_and_others = {Gelu, Tanh}); sigmoid is exact via
sigma(a) = (1 + tanh(a/2))/2, with 0.5 factors folded into weights:
  hz = tanh(-a_z/2), hr = tanh(+a_r/2)   (one ACT op; z top, r bottom)
  q  = (hr + 1) * hh        with hh = 0.5*(h_n + b_hn) -> q = r*(h_n+b_hn)
  w  = i_n + q              (PE identity-matmul accumulate into PSUM)
  nbar = tanh(-w - b_in) = -n
  d  = s + nbar = s - n
  u2 = (hz + 1) * d         (= 2*(1-z)*(s-n))
  s' = -0.5*u2 + s          (= z*s + (1-z)*n)
Elementwise tensors bf16; PSUM accumulation f32; outputs staged as f32
and DMA'd out every CH steps. The final [D, rows] -> [rows, D] transpose
happens on the host.
"""

import sys

import numpy as np

sys.path.insert(0, "/opt/trn_rl_repo")

import ml_dtypes  # noqa: E402

BF16 = ml_dtypes.bfloat16

T, B, C, D = 256, 64, 64, 64
NCORES = 8
ROWS = (B * C) // NCORES  # 512 rows per core
CH = 8  # timesteps per DMA chunk
NB = 4  # row blocks per core (pipelined independent chains)
BSIZES = [ROWS // NB] * NB
BOFF = [i * (ROWS // NB) for i in range(NB + 1)]

_PROGRAM_CACHE = {}


def _build_program():
    import concourse.bacc as bacc
    import concourse.tile as tile
    from concourse import mybir
    from contextlib import ExitStack

    BF = mybir.dt.bfloat16
    F32 = mybir.dt.float32
    AF = mybir.ActivationFunctionType
    OP = mybir.AluOpType

    # Bacc (not plain Bass): its compile() pass legalizes multi-semaphore
    # waits into event semaphores; raw Bass BIR trips walrus'
    # "Too many sync wait commands" on any instruction joining two streams.
    nc = bacc.Bacc(None, target_bir_lowering=False, debug=False)

    xT = nc.dram_tensor("xT", [T, D, ROWS], BF, kind="ExternalInput")
    s0 = nc.dram_tensor("s0", [D, ROWS], BF, kind="ExternalInput")
    tb = nc.dram_tensor("tb", [1, T, D], BF, kind="ExternalInput")
    # bf16 weights packed column-wise into one [D, 576] blob:
    #   wx [0:64], ws [64:128], wirz [128:256] (z cols first, then r),
    #   whrz [256:384], win [384:448], whn(0.5x) [448:512], prefh row0
    #   [512:576] (0.5*b_hn)
    wblob = nc.dram_tensor("wblob", [D, 9 * D], BF, kind="ExternalInput")
    # identity for the PE w-accumulate, at partitions 64:128
    iblob = nc.dram_tensor("iblob", [2 * D, D], BF, kind="ExternalInput")
    # f32 per-partition vectors [2D, 3]: col0 hrz scale (-0.5 | +0.5),
    # col1 hrz bias (-0.5*b_z | +0.5*b_r), col2 rows 0:64 = -b_in
    fblob = nc.dram_tensor("fblob", [2 * D, 3], F32, kind="ExternalInput")
    outT = nc.dram_tensor("outT", [T, D, ROWS], F32, kind="ExternalOutput")

    with ExitStack() as ctx:
        tc = ctx.enter_context(tile.TileContext(nc))
        consts = ctx.enter_context(tc.tile_pool(name="consts", bufs=1))
        xpool = ctx.enter_context(tc.tile_pool(name="xc", bufs=2))
        opool = ctx.enter_context(tc.tile_pool(name="ostage", bufs=2))
        spool = ctx.enter_context(tc.tile_pool(name="state", bufs=3))
        upool = ctx.enter_context(tc.tile_pool(name="u", bufs=2))
        gpool = ctx.enter_context(tc.tile_pool(name="gates", bufs=2))
        psum = ctx.enter_context(tc.tile_pool(name="psum", bufs=1, space="PSUM"))

        wblob_sb = consts.tile([D, 9 * D], BF, tag="wblob")
        nc.sync.dma_start(out=wblob_sb, in_=wblob[:, :])
        iblob_sb = consts.tile([2 * D, D], BF, tag="iblob")
        nc.sync.dma_start(out=iblob_sb, in_=iblob[:, :])
        fblob_sb = consts.tile([2 * D, 3], F32, tag="fblob")
        nc.sync.dma_start(out=fblob_sb, in_=fblob[:, :])
        tb_sb = consts.tile([1, T, D], BF, tag="tb")
        nc.sync.dma_start(out=tb_sb, in_=tb[:, :, :])

        wx_sb = wblob_sb[:, 0:D]
        ws_sb = wblob_sb[:, D : 2 * D]
        wirz_sb = wblob_sb[:, 2 * D : 4 * D]
        whrz_sb = wblob_sb[:, 4 * D : 6 * D]
        win_sb = wblob_sb[:, 6 * D : 7 * D]
        whn_sb = wblob_sb[:, 7 * D : 8 * D]
        prefh_sb = wblob_sb[0:1, 8 * D : 9 * D]
        ident_sb = iblob_sb[D : 2 * D, :]
        hrz_scale = fblob_sb[:, 0:1]
        hrz_bias = fblob_sb[:, 1:2]
        thbias_sb = fblob_sb[0:D, 2:3]
        ones_sb = consts.tile([1, ROWS], BF)
        nc.vector.memset(ones_sb, 1.0)

        # ACT allows few sync-waits; make the ACT engine observe the fblob
        # DMA lane once so per-step activations only need their PE/DVE wait.
        scratch = consts.tile([2 * D, 3], F32, tag="scratch")
        nc.scalar.copy(out=scratch, in_=fblob_sb)

        s_cur = []
        for b in range(NB):
            st = spool.tile([D, BSIZES[b]], BF, tag=f"state{b}")
            nc.sync.dma_start(out=st, in_=s0[:, BOFF[b] : BOFF[b + 1]])
            s_cur.append(st)

        xc = None
        ostage = None
        for t in range(T):
            k = t % CH
            if k == 0:
                xc = xpool.tile([D, CH, ROWS], BF, tag="xc")
                nc.sync.dma_start(
                    out=xc, in_=xT[t : t + CH, :, :].rearrange("c p r -> p c r")
                )
                ostage = opool.tile([D, CH, ROWS], F32, tag="ostage")

            for b in range(NB):
                rs = slice(BOFF[b], BOFF[b + 1])
                FDB = BSIZES[b]
                s_b = s_cur[b]

                # pmn bank: [0:D] holds the msg pre-activation, which gelu
                # consumes, then i_n overwrites it (start=True); [D:2D] holds
                # hh = 0.5*(h_n + b_hn). One PSUM bank per block for all of it.
                pmn = psum.tile([2 * D, FDB], F32, tag=f"pmn{b}")
                pm = pmn[0:D, :]
                nc.tensor.matmul(
                    pm, tb_sb[:, t, :], ones_sb[:, 0:FDB], start=True, stop=False
                )
                nc.tensor.matmul(pm, wx_sb, xc[:, k, rs], start=False, stop=False)
                nc.tensor.matmul(pm, ws_sb, s_b, start=False, stop=True)

                # u = gelu(pm)
                u = upool.tile([D, FDB], BF, tag=f"u{b}")
                nc.scalar.activation(u, pm, AF.Gelu)

                # rz gates pre-activation (z cols first, then r)
                prz = psum.tile([2 * D, FDB], F32, tag=f"prz{b}")
                nc.tensor.matmul(prz, wirz_sb, u, start=True, stop=False)
                nc.tensor.matmul(prz, whrz_sb, s_b, start=False, stop=True)

                pn = pmn
                nc.tensor.matmul(
                    pn[D : 2 * D, :],
                    prefh_sb,
                    ones_sb[:, 0:FDB],
                    start=True,
                    stop=False,
                )
                nc.tensor.matmul(pn[D : 2 * D, :], whn_sb, s_b, start=False, stop=True)
                # i_n overwrites the consumed msg region (start=True)
                nc.tensor.matmul(pn[0:D, :], win_sb, u, start=True, stop=False)

                # [hz; hr] = tanh(+-0.5 * a + b~)  (z top, r bottom)
                hrz = gpool.tile([2 * D, FDB], BF, tag=f"hrz{b}")
                nc.scalar.activation(
                    hrz, prz, AF.Tanh, bias=hrz_bias, scale=hrz_scale
                )

                # q = (hr + 1) * hh   (all at base partition 64)
                qt = gpool.tile([2 * D, FDB], BF, tag=f"q{b}")
                q = qt[D : 2 * D, :]
                nc.vector.scalar_tensor_tensor(
                    q, hrz[D : 2 * D, :], 1.0, pn[D : 2 * D, :], OP.add, OP.mult
                )
                # w = i_n + q via PE identity accumulate
                nc.tensor.matmul(pn[0:D, :], ident_sb, q, start=False, stop=True)
                # nbar = tanh(-w - b_in) = -n
                nbar = gpool.tile([D, FDB], BF, tag=f"nbar{b}")
                nc.scalar.activation(
                    nbar, pn[0:D, :], AF.Tanh, bias=thbias_sb, scale=-1.0
                )
                # d = s + nbar = s - n
                d = gpool.tile([D, FDB], BF, tag=f"d{b}")
                nc.vector.tensor_add(d, s_b, nbar)
                # u2 = (hz + 1) * d
                u2 = gpool.tile([D, FDB], BF, tag=f"u2{b}")
                nc.vector.scalar_tensor_tensor(
                    u2, hrz[0:D, :], 1.0, d, OP.add, OP.mult
                )
                # s' = -0.5*u2 + s
                s_nxt = spool.tile([D, FDB], BF, tag=f"state{b}")
                nc.vector.scalar_tensor_tensor(
                    s_nxt, u2, -0.5, s_b, OP.mult, OP.add
                )
                # stage output (bf16 -> f32 upcast) off the ACT/DVE engines
                nc.gpsimd.tensor_copy(out=ostage[:, k, rs], in_=s_nxt)
                s_cur[b] = s_nxt

            if k == CH - 1:
                nc.sync.dma_start(
                    out=outT[t - CH + 1 : t + 1, :, :].rearrange("c p r -> p c r"),
                    in_=ostage,
                )

    nc.compile()
    return nc


def _prep_host(x, mask, msg_W, msg_b, W_ih, W_hh, b_ih, b_hh, basis_freq, phase):
    """Host-side prep: sharding/layout + tiny weight preprocessing."""
    x = np.asarray(x, dtype=np.float32)
    mask = np.asarray(mask)
    msg_W = np.asarray(msg_W, np.float32)
    msg_b = np.asarray(msg_b, np.float32)
    W_ih = np.asarray(W_ih, np.float32)
    W_hh = np.asarray(W_hh, np.float32)
    b_ih = np.asarray(b_ih, np.float32)
    b_hh = np.asarray(b_hh, np.float32)
    basis_freq = np.asarray(basis_freq, np.float32)
    phase = np.asarray(phase, np.float32)

    tr = np.arange(T, dtype=np.int64) * mask.astype(np.int64)
    identity_gather = bool(np.array_equal(tr, np.arange(T)))

    xf = x.reshape(T, B * C, D)
    s0_rows = xf.mean(axis=0)  # [B*C, D] f32 (from ungathered x)
    if not identity_gather:
        xf = xf[tr]

    x4 = xf.reshape(T, NCORES, ROWS, D)
    xT8 = [
        np.ascontiguousarray(x4[:, c].transpose(0, 2, 1)).astype(BF16)
        for c in range(NCORES)
    ]
    s08 = [
        np.ascontiguousarray(s0_rows[c * ROWS : (c + 1) * ROWS].T).astype(BF16)
        for c in range(NCORES)
    ]

    ts_ = np.arange(T, dtype=np.float32)[tr]
    te = np.cos(ts_[:, None] * basis_freq[None, :] + phase[None, :])  # [T, D]
    Wt = msg_W[:, 2 * D : 3 * D]
    tb_host = (te @ Wt.T + msg_b[None, :]).astype(BF16).reshape(1, T, D)

    wblob = np.zeros((D, 9 * D), np.float32)
    wblob[:, 0:D] = msg_W[:, 0:D].T
    wblob[:, D : 2 * D] = msg_W[:, D : 2 * D].T
    # z gate columns first, then r (matches hz-top/hr-bottom ACT layout)
    wblob[:, 2 * D : 3 * D] = W_ih[D : 2 * D].T
    wblob[:, 3 * D : 4 * D] = W_ih[0:D].T
    wblob[:, 4 * D : 5 * D] = W_hh[D : 2 * D].T
    wblob[:, 5 * D : 6 * D] = W_hh[0:D].T
    wblob[:, 6 * D : 7 * D] = W_ih[2 * D : 3 * D].T
    wblob[:, 7 * D : 8 * D] = 0.5 * W_hh[2 * D : 3 * D].T
    wblob[0, 8 * D : 9 * D] = 0.5 * b_hh[2 * D : 3 * D]

    iblob = np.zeros((2 * D, D), np.float32)
    iblob[D : 2 * D, :] = np.eye(D, dtype=np.float32)

    fblob = np.zeros((2 * D, 3), np.float32)
    fblob[0:D, 0] = -0.5
    fblob[D : 2 * D, 0] = 0.5
    fblob[0:D, 1] = -0.5 * (b_ih[D : 2 * D] + b_hh[D : 2 * D])
    fblob[D : 2 * D, 1] = 0.5 * (b_ih[0:D] + b_hh[0:D])
    fblob[0:D, 2] = -b_ih[2 * D : 3 * D]

    shared = {
        "tb": tb_host,
        "wblob": wblob.astype(BF16),
        "iblob": iblob.astype(BF16),
        "fblob": fblob,
    }
    in_maps = []
    for c in range(NCORES):
        m = dict(shared)
        m["xT"] = xT8[c]
        m["s0"] = s08[c]
        in_maps.append(m)
    return in_maps


def kernel(**inputs):
    from concourse.bass_utils import run_bass_kernel_spmd

    in_maps = _prep_host(**inputs)

    if "prog" not in _PROGRAM_CACHE:
        _PROGRAM_CACHE["prog"] = _build_program()
    nc = _PROGRAM_CACHE["prog"]

    res = run_bass_kernel_spmd(nc, in_maps, core_ids=list(range(NCORES)))
    _PROGRAM_CACHE["last_results"] = res

    out = np.empty((T, B * C, D), dtype=np.float32)
    for c in range(NCORES):
        outT_c = res.results[c]["outT"]  # [T, D, ROWS] f32
        out[:, c * ROWS : (c + 1) * ROWS, :] = outT_c.transpose(0, 2, 1)
    return out.reshape(T, B, C, D)



# revision 5
# speedup vs baseline: 1.0296x; 1.0296x over previous
"""Trainium2 Bass kernel for nn_MemoryNetwork (GRU-style memory network scan).

Model (per reference):
  t_enc = cos(arange(T) * freq + phase)                    [T, D]
  s0 = mean_t(x)                                           [B*C, D]
  tr = arange(T) * mask; x_seq = x[tr]; te_seq = t_enc[tr]
  per step t:
    msg = gelu([x_t, s, te_t] @ msg_W.T + msg_b)
    gi = msg @ W_ih.T + b_ih ; gh = s @ W_hh.T + b_hh
    r = sigmoid(i_r + h_r); z = sigmoid(i_z + h_z)
    n = tanh(i_n + r * h_n)
    s' = (1 - z) * n + z * s
  output: states [T, B, C, D]

Strategy: data-parallel over B*C = 4096 rows -> 8 cores x 512 rows.
On-device layout is feature-major and PARTITION-PACKED: two rows share a
packed column (row c on partitions 0:64, row 256+c on partitions 64:128),
so every instruction covers two rows' features at once.  All weights are
block-diagonal doubled [128, 128]; matmul cost depends only on the free
dim, so packing halves the activation-engine work (the bottleneck) for
free.  The 256 packed columns split into NPB=3 independent chains
(86/85/85) whose per-step dependency chains interleave to hide the
serial-scan latency.

The activation engine runs exactly 3 ops per chain-step, all with
scale=1/bias=0 (every gate scale/bias is folded into the block-diagonal
weights or added in PSUM via broadcast-row matmuls):
  u   = gelu(pm),            pm = tb_t + Wx@x + Ws@s          [128, F]
  hzr = tanh(zr),            zr = [-a_z/2 | +a_r/2]           [128, 2F]
  n   = tanh(pw),            pw = b_in + W_in@u + I@q         [128, F]
with q = (hr + 1) * hh, hh = 0.5*(W_hn@s + b_hn) computed on the Pool
engine, and the GRU blend on DVE:
  d = s - n;  u2 = (hz + 1) * d;  s' = -0.5*u2 + s
(z = (1 - hz)/2 with hz = tanh(-a_z/2), r = (1 + hr)/2 with
hr = tanh(+a_r/2), so s' = (1-z)*n + z*s exactly.)

Each chain-step owns one PSUM bank [128, 512] f32, double-buffered:
cols [0:F) msg pre-act, [F:3F) z|r pre-acts, [3F:4F) n pre-act,
[4F:5F) hh.  Outputs stream out as bf16 every CH steps; the host
unpacks partitions back to rows.
"""

import sys

import numpy as np

sys.path.insert(0, "/opt/trn_rl_repo")

import ml_dtypes  # noqa: E402

BF16 = ml_dtypes.bfloat16

T, B, C, D = 256, 64, 64, 64
NCORES = 8
ROWS = (B * C) // NCORES  # 512 rows per core
HALF = ROWS // 2  # 256 packed columns (2 rows per column)
CH = 8  # timesteps per DMA chunk
NPB = 3  # packed chains per core
PF = [86, 85, 85]  # packed columns per chain
POFF = [0, 86, 171, 256]

_PROGRAM_CACHE = {}


def _build_program():
    import concourse.bacc as bacc
    import concourse.tile as tile
    from concourse import mybir
    from contextlib import ExitStack

    BF = mybir.dt.bfloat16
    F32 = mybir.dt.float32
    AF = mybir.ActivationFunctionType
    OP = mybir.AluOpType

    # Bacc (not plain Bass): its compile() pass legalizes multi-semaphore
    # waits into event semaphores.
    nc = bacc.Bacc(None, target_bir_lowering=False, debug=False)

    xP = nc.dram_tensor("xP", [T, 2 * D, HALF], BF, kind="ExternalInput")
    s0 = nc.dram_tensor("s0", [2 * D, HALF], BF, kind="ExternalInput")
    tb = nc.dram_tensor("tb", [1, T, 2 * D], BF, kind="ExternalInput")
    # 8 block-diagonal doubled weight mats + identity, packed [128, 9*128]:
    #   wx, ws, wiz(-.5), whz(-.5), wir(+.5), whr(+.5), win, whn(.5), I
    wblob = nc.dram_tensor("wblob", [2 * D, 9 * 2 * D], BF, kind="ExternalInput")
    # bias rows on partition 0, side by side [1, 4*128]:
    # -0.5(b_iz+b_hz) | +0.5(b_ir+b_hr) | b_in | 0.5*b_hn  (each tiled 2x)
    bblob = nc.dram_tensor("bblob", [1, 4 * 2 * D], BF, kind="ExternalInput")
    outP = nc.dram_tensor("outP", [T, 2 * D, HALF], BF, kind="ExternalOutput")

    DD = 2 * D  # 128

    with ExitStack() as ctx:
        tc = ctx.enter_context(tile.TileContext(nc))
        consts = ctx.enter_context(tc.tile_pool(name="consts", bufs=1))
        xpool = ctx.enter_context(tc.tile_pool(name="xc", bufs=2))
        opool = ctx.enter_context(tc.tile_pool(name="ostage", bufs=2))
        spool = ctx.enter_context(tc.tile_pool(name="state", bufs=3))
        upool = ctx.enter_context(tc.tile_pool(name="u", bufs=2))
        gpool = ctx.enter_context(tc.tile_pool(name="gates", bufs=2))
        psum = ctx.enter_context(tc.tile_pool(name="psum", bufs=2, space="PSUM"))

        wblob_sb = consts.tile([DD, 9 * DD], BF, tag="wblob")
        nc.sync.dma_start(out=wblob_sb, in_=wblob[:, :])
        bblob_sb = consts.tile([1, 4 * DD], BF, tag="bblob")
        nc.sync.dma_start(out=bblob_sb, in_=bblob[:, :])
        tb_sb = consts.tile([1, T, DD], BF, tag="tb")
        nc.sync.dma_start(out=tb_sb, in_=tb[:, :, :])

        wx_sb = wblob_sb[:, 0 * DD : 1 * DD]
        ws_sb = wblob_sb[:, 1 * DD : 2 * DD]
        wiz_sb = wblob_sb[:, 2 * DD : 3 * DD]
        whz_sb = wblob_sb[:, 3 * DD : 4 * DD]
        wir_sb = wblob_sb[:, 4 * DD : 5 * DD]
        whr_sb = wblob_sb[:, 5 * DD : 6 * DD]
        win_sb = wblob_sb[:, 6 * DD : 7 * DD]
        whn_sb = wblob_sb[:, 7 * DD : 8 * DD]
        ident_sb = wblob_sb[:, 8 * DD : 9 * DD]
        bz_sb = bblob_sb[0:1, 0 * DD : 1 * DD]
        br_sb = bblob_sb[0:1, 1 * DD : 2 * DD]
        bin_sb = bblob_sb[0:1, 2 * DD : 3 * DD]
        bhn_sb = bblob_sb[0:1, 3 * DD : 4 * DD]
        ones_sb = consts.tile([1, HALF], BF)
        nc.vector.memset(ones_sb, 1.0)

        s_cur = []
        for p in range(NPB):
            st = spool.tile([DD, PF[p]], BF, tag=f"state{p}")
            nc.sync.dma_start(out=st, in_=s0[:, POFF[p] : POFF[p + 1]])
            s_cur.append(st)

        xc = None
        ostage = None
        for t in range(T):
            k = t % CH
            if k == 0:
                xc = xpool.tile([DD, CH, HALF], BF, tag="xc")
                nc.sync.dma_start(
                    out=xc, in_=xP[t : t + CH, :, :].rearrange("c p r -> p c r")
                )
                ostage = opool.tile([DD, CH, HALF], BF, tag="ostage")

            for p in range(NPB):
                F = PF[p]
                cs = slice(POFF[p], POFF[p + 1])
                s_b = s_cur[p]

                bank = psum.tile([DD, 512], F32, tag=f"bank{p}")
                pm = bank[:, 0:F]
                zr = bank[:, F : 3 * F]
                zz = bank[:, F : 2 * F]
                rr = bank[:, 2 * F : 3 * F]
                pw = bank[:, 3 * F : 4 * F]
                hh = bank[:, 4 * F : 5 * F]

                # pm = tb_t + Wx@x + Ws@s
                nc.tensor.matmul(
                    pm, tb_sb[:, t, :], ones_sb[:, 0:F], start=True, stop=False
                )
                nc.tensor.matmul(pm, wx_sb, xc[:, k, cs], start=False, stop=False)
                nc.tensor.matmul(pm, ws_sb, s_b, start=False, stop=False)

                u = upool.tile([DD, F], BF, tag=f"u{p}")
                nc.scalar.activation(u, pm, AF.Gelu)

                # zr pre-acts: [-a_z/2 | +a_r/2] (scales/biases folded)
                nc.tensor.matmul(zz, bz_sb, ones_sb[:, 0:F], start=False, stop=False)
                nc.tensor.matmul(rr, br_sb, ones_sb[:, 0:F], start=False, stop=False)
                nc.tensor.matmul(zz, whz_sb, s_b, start=False, stop=False)
                nc.tensor.matmul(rr, whr_sb, s_b, start=False, stop=False)
                nc.tensor.matmul(zz, wiz_sb, u, start=False, stop=False)
                nc.tensor.matmul(rr, wir_sb, u, start=False, stop=False)

                hzr = gpool.tile([DD, 2 * F], BF, tag=f"hzr{p}")
                nc.scalar.activation(hzr, zr, AF.Tanh)
                hz = hzr[:, 0:F]
                hr = hzr[:, F : 2 * F]

                # hh = 0.5*(W_hn@s + b_hn);  pw = b_in + W_in@u (+ I@q below)
                nc.tensor.matmul(hh, bhn_sb, ones_sb[:, 0:F], start=False, stop=False)
                nc.tensor.matmul(hh, whn_sb, s_b, start=False, stop=False)
                nc.tensor.matmul(pw, bin_sb, ones_sb[:, 0:F], start=False, stop=False)
                nc.tensor.matmul(pw, win_sb, u, start=False, stop=False)

                # q = (hr + 1) * hh   (DVE: GPSIMD cannot access PSUM)
                q = gpool.tile([DD, F], BF, tag=f"q{p}")
                nc.vector.scalar_tensor_tensor(q, hr, 1.0, hh, OP.add, OP.mult)
                nc.tensor.matmul(pw, ident_sb, q, start=False, stop=True)

                n = gpool.tile([DD, F], BF, tag=f"n{p}")
                nc.scalar.activation(n, pw, AF.Tanh)

                # d = s - n; u2 = (hz+1)*d; s' = -0.5*u2 + s
                d = gpool.tile([DD, F], BF, tag=f"d{p}")
                nc.vector.tensor_sub(d, s_b, n)
                u2 = gpool.tile([DD, F], BF, tag=f"u2{p}")
                nc.vector.scalar_tensor_tensor(u2, hz, 1.0, d, OP.add, OP.mult)
                s_nxt = spool.tile([DD, F], BF, tag=f"state{p}")
                nc.vector.scalar_tensor_tensor(
                    s_nxt, u2, -0.5, s_b, OP.mult, OP.add
                )
                nc.gpsimd.tensor_copy(out=ostage[:, k, cs], in_=s_nxt)
                s_cur[p] = s_nxt

            if k == CH - 1:
                nc.sync.dma_start(
                    out=outP[t - CH + 1 : t + 1, :, :].rearrange("c p r -> p c r"),
                    in_=ostage,
                )

    nc.compile()
    return nc


def _blkdiag(a):
    """[64, 64] -> [128, 128] block-diagonal double, bf16."""
    out = np.zeros((2 * D, 2 * D), np.float32)
    out[:D, :D] = a
    out[D:, D:] = a
    return out


def _prep_host(x, mask, msg_W, msg_b, W_ih, W_hh, b_ih, b_hh, basis_freq, phase):
    """Host-side prep: partition-packing, sharding, weight doubling."""
    x = np.asarray(x, dtype=np.float32)
    mask = np.asarray(mask)
    msg_W = np.asarray(msg_W, np.float32)
    msg_b = np.asarray(msg_b, np.float32)
    W_ih = np.asarray(W_ih, np.float32)
    W_hh = np.asarray(W_hh, np.float32)
    b_ih = np.asarray(b_ih, np.float32)
    b_hh = np.asarray(b_hh, np.float32)
    basis_freq = np.asarray(basis_freq, np.float32)
    phase = np.asarray(phase, np.float32)

    tr = np.arange(T, dtype=np.int64) * mask.astype(np.int64)
    identity_gather = bool(np.array_equal(tr, np.arange(T)))

    xf = x.reshape(T, B * C, D)
    s0_rows = xf.mean(axis=0)  # [B*C, D] f32 (from ungathered x)
    if not identity_gather:
        xf = xf[tr]

    xP8, s08 = [], []
    for c in range(NCORES):
        blk = xf[:, c * ROWS : (c + 1) * ROWS, :]  # [T, 512, 64]
        lo = blk[:, 0:HALF].transpose(0, 2, 1)  # [T, 64, 256]
        hi = blk[:, HALF:ROWS].transpose(0, 2, 1)
        xP8.append(np.ascontiguousarray(
            np.concatenate([lo, hi], axis=1)).astype(BF16))
        sblk = s0_rows[c * ROWS : (c + 1) * ROWS]  # [512, 64]
        s08.append(np.ascontiguousarray(np.concatenate(
            [sblk[0:HALF].T, sblk[HALF:ROWS].T], axis=0)).astype(BF16))

    ts_ = np.arange(T, dtype=np.float32)[tr]
    te = np.cos(ts_[:, None] * basis_freq[None, :] + phase[None, :])  # [T, D]
    Wt = msg_W[:, 2 * D : 3 * D]
    tb1 = te @ Wt.T + msg_b[None, :]  # [T, 64]
    tb_host = np.tile(tb1, (1, 2)).astype(BF16).reshape(1, T, 2 * D)

    Wx = msg_W[:, 0:D].T
    Ws = msg_W[:, D : 2 * D].T
    # torch gate order in W_ih/W_hh: rows [r, z, n]
    Wir, Wiz, Win = W_ih[0:D], W_ih[D : 2 * D], W_ih[2 * D : 3 * D]
    Whr, Whz, Whn = W_hh[0:D], W_hh[D : 2 * D], W_hh[2 * D : 3 * D]

    mats = [
        Wx, Ws,
        -0.5 * Wiz.T, -0.5 * Whz.T,
        0.5 * Wir.T, 0.5 * Whr.T,
        Win.T, 0.5 * Whn.T,
        np.eye(D, dtype=np.float32),
    ]
    wblob = np.concatenate([_blkdiag(m) for m in mats], axis=1)

    bblob = np.concatenate([
        np.tile(-0.5 * (b_ih[D : 2 * D] + b_hh[D : 2 * D]), 2),
        np.tile(0.5 * (b_ih[0:D] + b_hh[0:D]), 2),
        np.tile(b_ih[2 * D : 3 * D], 2),
        np.tile(0.5 * b_hh[2 * D : 3 * D], 2),
    ]).reshape(1, 4 * 2 * D)

    shared = {
        "tb": tb_host,
        "wblob": wblob.astype(BF16),
        "bblob": bblob.astype(BF16),
    }
    in_maps = []
    for c in range(NCORES):
        m = dict(shared)
        m["xP"] = xP8[c]
        m["s0"] = s08[c]
        in_maps.append(m)
    return in_maps


def kernel(**inputs):
    from concourse.bass_utils import run_bass_kernel_spmd

    in_maps = _prep_host(**inputs)

    if "prog" not in _PROGRAM_CACHE:
        _PROGRAM_CACHE["prog"] = _build_program()
    nc = _PROGRAM_CACHE["prog"]

    res = run_bass_kernel_spmd(nc, in_maps, core_ids=list(range(NCORES)))
    _PROGRAM_CACHE["last_results"] = res

    out = np.empty((T, B * C, D), dtype=np.float32)
    for c in range(NCORES):
        outP_c = np.asarray(res.results[c]["outP"], dtype=np.float32)  # [T,128,256]
        base = c * ROWS
        out[:, base : base + HALF, :] = outP_c[:, 0:D, :].transpose(0, 2, 1)
        out[:, base + HALF : base + ROWS, :] = outP_c[:, D:, :].transpose(0, 2, 1)
    return out.reshape(T, B, C, D)


# revision 7
# speedup vs baseline: 1.2031x; 1.1685x over previous
"""Trainium2 Bass kernel for nn_MemoryNetwork (GRU-style memory network scan).

Model (per reference):
  t_enc = cos(arange(T) * freq + phase)                    [T, D]
  s0 = mean_t(x)                                           [B*C, D]
  tr = arange(T) * mask; x_seq = x[tr]; te_seq = t_enc[tr]
  per step t:
    msg = gelu([x_t, s, te_t] @ msg_W.T + msg_b)
    gi = msg @ W_ih.T + b_ih ; gh = s @ W_hh.T + b_hh
    r = sigmoid(i_r + h_r); z = sigmoid(i_z + h_z)
    n = tanh(i_n + r * h_n)
    s' = (1 - z) * n + z * s
  output: states [T, B, C, D]

Strategy: data-parallel over B*C = 4096 rows -> 8 cores x 512 rows.
On-device layout is feature-major and PARTITION-PACKED: two rows share a
packed column (row c on partitions 0:64, row 256+c on partitions 64:128),
with block-diagonal doubled weights [128, 128].  Matmul cost depends only
on the free dim, so packing halves the activation-engine work (the
bottleneck) for free.  The 256 packed columns split into NPB=3
independent chains (86/85/85).

Engines execute their queues IN PROGRAM ORDER, so the emission order is
SOFTWARE-PIPELINED: chain p runs phase (slot - p) % 3 of its step at each
slot, staggering the three chains across the three ACT ops of a step
(gelu / zr-tanh / n-tanh).  Between two consecutive ACT ops of one chain
the ACT engine executes the other two chains' (independent) ops, hiding
each chain's PE/DVE dependency latency.  Phases:

  A(t): s-dependent matmuls (Ws@s into pm; Whz/Whr@s + z/r bias rows into
        zr; Whn@s + bias into hh; b_in into pw), then u = gelu(pm).
  B(t): u-side matmuls (Wiz/Wir@u, Win@u), hzr = tanh([-a_z/2 | +a_r/2]),
        q = (hr+1)*hh on DVE, I@q accumulated into pw (PE),
        Q = 0.5*hz+0.5, Q' = -0.5*hz+0.5 (= 1-z, z), w1 = Q'*s on DVE.
  C(t): n = tanh(pw), v = Q*n, s' = v + w1 (= (1-z)*n + z*s), Pool-copy
        s' to the output stage, and prehoist step t+1's x-dependent
        matmuls (tb broadcast row with start=True, Wx@x) into a fresh
        PSUM bank.

All gate scales/biases are folded into the doubled weights or added in
PSUM via broadcast-row matmuls, so every ACT op is plain func(x).  Each
chain-step owns one PSUM bank [128, 512] f32 (one start=True / one
stop=True per bank: start zeroes the whole 2KB zero-region).  Outputs
stream out as bf16 every CH steps; the host unpacks partitions to rows.
"""

import sys

import numpy as np

sys.path.insert(0, "/opt/trn_rl_repo")

import ml_dtypes  # noqa: E402

BF16 = ml_dtypes.bfloat16

T, B, C, D = 256, 64, 64, 64
NCORES = 8
ROWS = (B * C) // NCORES  # 512 rows per core
HALF = ROWS // 2  # 256 packed columns (2 rows per column)
CH = 8  # timesteps per DMA chunk
NPB = 3  # packed chains per core
PF = [86, 85, 85]  # packed columns per chain
POFF = [0, 86, 171, 256]

_PROGRAM_CACHE = {}


def _build_program():
    import concourse.bacc as bacc
    import concourse.tile as tile
    from concourse import mybir
    from contextlib import ExitStack

    BF = mybir.dt.bfloat16
    F32 = mybir.dt.float32
    AF = mybir.ActivationFunctionType
    OP = mybir.AluOpType

    nc = bacc.Bacc(None, target_bir_lowering=False, debug=False)

    xP = nc.dram_tensor("xP", [T, 2 * D, HALF], BF, kind="ExternalInput")
    s0 = nc.dram_tensor("s0", [2 * D, HALF], BF, kind="ExternalInput")
    tb = nc.dram_tensor("tb", [1, T, 2 * D], BF, kind="ExternalInput")
    # 8 block-diagonal doubled weight mats + identity, packed [128, 9*128]:
    #   wx, ws, wiz(-.5), whz(-.5), wir(+.5), whr(+.5), win, whn(.5), I
    wblob = nc.dram_tensor("wblob", [2 * D, 9 * 2 * D], BF, kind="ExternalInput")
    # bias rows on partition 0, side by side [1, 4*128]:
    # -0.5(b_iz+b_hz) | +0.5(b_ir+b_hr) | b_in | 0.5*b_hn  (each tiled 2x)
    bblob = nc.dram_tensor("bblob", [1, 4 * 2 * D], BF, kind="ExternalInput")
    outP = nc.dram_tensor("outP", [T, 2 * D, HALF], BF, kind="ExternalOutput")

    DD = 2 * D  # 128

    with ExitStack() as ctx:
        tc = ctx.enter_context(tile.TileContext(nc))
        consts = ctx.enter_context(tc.tile_pool(name="consts", bufs=1))
        xpool = ctx.enter_context(tc.tile_pool(name="xc", bufs=2))
        opool = ctx.enter_context(tc.tile_pool(name="ostage", bufs=2))
        spool = ctx.enter_context(tc.tile_pool(name="state", bufs=3))
        upool = ctx.enter_context(tc.tile_pool(name="u", bufs=2))
        gpool = ctx.enter_context(tc.tile_pool(name="gates", bufs=2))
        psum = ctx.enter_context(tc.tile_pool(name="psum", bufs=2, space="PSUM"))

        wblob_sb = consts.tile([DD, 9 * DD], BF, tag="wblob")
        nc.sync.dma_start(out=wblob_sb, in_=wblob[:, :])
        bblob_sb = consts.tile([1, 4 * DD], BF, tag="bblob")
        nc.sync.dma_start(out=bblob_sb, in_=bblob[:, :])
        tb_sb = consts.tile([1, T, DD], BF, tag="tb")
        nc.sync.dma_start(out=tb_sb, in_=tb[:, :, :])

        wx_sb = wblob_sb[:, 0 * DD : 1 * DD]
        ws_sb = wblob_sb[:, 1 * DD : 2 * DD]
        wiz_sb = wblob_sb[:, 2 * DD : 3 * DD]
        whz_sb = wblob_sb[:, 3 * DD : 4 * DD]
        wir_sb = wblob_sb[:, 4 * DD : 5 * DD]
        whr_sb = wblob_sb[:, 5 * DD : 6 * DD]
        win_sb = wblob_sb[:, 6 * DD : 7 * DD]
        whn_sb = wblob_sb[:, 7 * DD : 8 * DD]
        ident_sb = wblob_sb[:, 8 * DD : 9 * DD]
        bz_sb = bblob_sb[0:1, 0 * DD : 1 * DD]
        br_sb = bblob_sb[0:1, 1 * DD : 2 * DD]
        bin_sb = bblob_sb[0:1, 2 * DD : 3 * DD]
        bhn_sb = bblob_sb[0:1, 3 * DD : 4 * DD]
        ones_sb = consts.tile([1, HALF], BF)
        nc.vector.memset(ones_sb, 1.0)

        # --- pipeline state per chain ---
        R = []
        for p in range(NPB):
            st = spool.tile([DD, PF[p]], BF, tag=f"state{p}")
            nc.sync.dma_start(out=st, in_=s0[:, POFF[p] : POFF[p + 1]])
            R.append({"s": st})

        chunks = {}  # chunk idx -> xc tile
        ostages = {}  # chunk idx -> ostage tile

        def get_chunk(c):
            if c not in chunks:
                t0 = c * CH
                xc = xpool.tile([DD, CH, HALF], BF, tag="xc", name="xc")
                nc.sync.dma_start(
                    out=xc, in_=xP[t0 : t0 + CH, :, :].rearrange("c p r -> p c r")
                )
                chunks[c] = xc
            return chunks[c]

        def get_ostage(c):
            if c not in ostages:
                ostages[c] = opool.tile([DD, CH, HALF], BF, tag="ostage", name="ostage")
            return ostages[c]

        def regions(bank, F):
            return (bank[:, 0:F], bank[:, F : 3 * F], bank[:, F : 2 * F],
                    bank[:, 2 * F : 3 * F], bank[:, 3 * F : 4 * F],
                    bank[:, 4 * F : 5 * F])

        def prehoist(p, t):
            """Fresh PSUM bank for (p, t): tb broadcast (start=True, zeroes
            the bank) + Wx@x.  Emitted one phase before A(t)."""
            F = PF[p]
            cs = slice(POFF[p], POFF[p + 1])
            bank = psum.tile([DD, 512], F32, tag=f"bank{p}", name=f"bank{p}")
            pm = bank[:, 0:F]
            nc.tensor.matmul(
                pm, tb_sb[:, t, :], ones_sb[:, 0:F], start=True, stop=False
            )
            xc = get_chunk(t // CH)
            nc.tensor.matmul(pm, wx_sb, xc[:, t % CH, cs], start=False, stop=False)
            R[p]["bank"] = bank

        def phaseA(p, t):
            F = PF[p]
            s_b = R[p]["s"]
            bank = R[p]["bank"]
            pm, zr, zz, rr, pw, hh = regions(bank, F)
            nc.tensor.matmul(pm, ws_sb, s_b, start=False, stop=False)
            nc.tensor.matmul(zz, bz_sb, ones_sb[:, 0:F], start=False, stop=False)
            nc.tensor.matmul(rr, br_sb, ones_sb[:, 0:F], start=False, stop=False)
            nc.tensor.matmul(zz, whz_sb, s_b, start=False, stop=False)
            nc.tensor.matmul(rr, whr_sb, s_b, start=False, stop=False)
            nc.tensor.matmul(hh, bhn_sb, ones_sb[:, 0:F], start=False, stop=False)
            nc.tensor.matmul(hh, whn_sb, s_b, start=False, stop=False)
            nc.tensor.matmul(pw, bin_sb, ones_sb[:, 0:F], start=False, stop=False)
            u = upool.tile([DD, F], BF, tag=f"u{p}", name=f"u{p}")
            nc.scalar.activation(u, pm, AF.Gelu)
            R[p]["u"] = u

        def phaseB(p, t):
            F = PF[p]
            s_b = R[p]["s"]
            bank = R[p]["bank"]
            pm, zr, zz, rr, pw, hh = regions(bank, F)
            u = R[p]["u"]
            nc.tensor.matmul(zz, wiz_sb, u, start=False, stop=False)
            nc.tensor.matmul(rr, wir_sb, u, start=False, stop=False)
            nc.tensor.matmul(pw, win_sb, u, start=False, stop=False)
            hzr = gpool.tile([DD, 2 * F], BF, tag=f"hzr{p}", name=f"hzr{p}")
            nc.scalar.activation(hzr, zr, AF.Tanh)
            hz = hzr[:, 0:F]
            hr = hzr[:, F : 2 * F]
            # q = (hr + 1) * hh  (DVE; GPSIMD cannot access PSUM)
            q = gpool.tile([DD, F], BF, tag=f"q{p}", name=f"q{p}")
            nc.vector.scalar_tensor_tensor(q, hr, 1.0, hh, OP.add, OP.mult)
            nc.tensor.matmul(pw, ident_sb, q, start=False, stop=True)
            # blend coefficients: Q = 1-z, Q' = z;  w1 = z*s
            Q = gpool.tile([DD, F], BF, tag=f"Q{p}", name=f"Qc{p}")
            nc.vector.tensor_scalar(Q, hz, 0.5, 0.5, OP.mult, OP.add)
            Qp = gpool.tile([DD, F], BF, tag=f"Qp{p}", name=f"Qp{p}")
            nc.vector.tensor_scalar(Qp, hz, -0.5, 0.5, OP.mult, OP.add)
            w1 = gpool.tile([DD, F], BF, tag=f"w1{p}", name=f"w1{p}")
            nc.vector.tensor_mul(w1, Qp, s_b)
            R[p]["Q"] = Q
            R[p]["w1"] = w1

        def phaseC(p, t):
            F = PF[p]
            cs = slice(POFF[p], POFF[p + 1])
            bank = R[p]["bank"]
            pw = bank[:, 3 * F : 4 * F]
            n = gpool.tile([DD, F], BF, tag=f"n{p}", name=f"n{p}")
            nc.scalar.activation(n, pw, AF.Tanh)
            v = gpool.tile([DD, F], BF, tag=f"v{p}", name=f"v{p}")
            nc.vector.tensor_mul(v, R[p]["Q"], n)
            s_nxt = spool.tile([DD, F], BF, tag=f"state{p}", name=f"state{p}")
            nc.vector.tensor_add(s_nxt, v, R[p]["w1"])
            ost = get_ostage(t // CH)
            nc.gpsimd.tensor_copy(out=ost[:, t % CH, cs], in_=s_nxt)
            R[p]["s"] = s_nxt
            if t + 1 < T:
                prehoist(p, t + 1)
            if p == NPB - 1 and t % CH == CH - 1:
                c = t // CH
                nc.sync.dma_start(
                    out=outP[t - CH + 1 : t + 1, :, :].rearrange("c p r -> p c r"),
                    in_=ostages.pop(c),
                )
                chunks.pop(c, None)

        # --- bootstrap: bank(0) + x-mms for every chain ---
        for p in range(NPB):
            prehoist(p, 0)

        # --- pipelined slot loop: chain p does phase (i - p) % 3 ---
        PHASES = [phaseA, phaseB, phaseC]
        for i in range(3 * T + 2):
            for p in range(NPB):
                ph = (i - p) % 3
                t = (i - p) // 3
                if 0 <= t < T:
                    PHASES[ph](p, t)

    nc.compile()
    return nc


def _blkdiag(a):
    """[64, 64] -> [128, 128] block-diagonal double."""
    out = np.zeros((2 * D, 2 * D), np.float32)
    out[:D, :D] = a
    out[D:, D:] = a
    return out


def _prep_host(x, mask, msg_W, msg_b, W_ih, W_hh, b_ih, b_hh, basis_freq, phase):
    """Host-side prep: partition-packing, sharding, weight doubling."""
    x = np.asarray(x, dtype=np.float32)
    mask = np.asarray(mask)
    msg_W = np.asarray(msg_W, np.float32)
    msg_b = np.asarray(msg_b, np.float32)
    W_ih = np.asarray(W_ih, np.float32)
    W_hh = np.asarray(W_hh, np.float32)
    b_ih = np.asarray(b_ih, np.float32)
    b_hh = np.asarray(b_hh, np.float32)
    basis_freq = np.asarray(basis_freq, np.float32)
    phase = np.asarray(phase, np.float32)

    tr = np.arange(T, dtype=np.int64) * mask.astype(np.int64)
    identity_gather = bool(np.array_equal(tr, np.arange(T)))

    xf = x.reshape(T, B * C, D)
    s0_rows = xf.mean(axis=0)  # [B*C, D] f32 (from ungathered x)
    if not identity_gather:
        xf = xf[tr]

    xP8, s08 = [], []
    for c in range(NCORES):
        blk = xf[:, c * ROWS : (c + 1) * ROWS, :]  # [T, 512, 64]
        lo = blk[:, 0:HALF].transpose(0, 2, 1)  # [T, 64, 256]
        hi = blk[:, HALF:ROWS].transpose(0, 2, 1)
        xP8.append(np.ascontiguousarray(
            np.concatenate([lo, hi], axis=1)).astype(BF16))
        sblk = s0_rows[c * ROWS : (c + 1) * ROWS]  # [512, 64]
        s08.append(np.ascontiguousarray(np.concatenate(
            [sblk[0:HALF].T, sblk[HALF:ROWS].T], axis=0)).astype(BF16))

    ts_ = np.arange(T, dtype=np.float32)[tr]
    te = np.cos(ts_[:, None] * basis_freq[None, :] + phase[None, :])  # [T, D]
    Wt = msg_W[:, 2 * D : 3 * D]
    tb1 = te @ Wt.T + msg_b[None, :]  # [T, 64]
    tb_host = np.tile(tb1, (1, 2)).astype(BF16).reshape(1, T, 2 * D)

    Wx = msg_W[:, 0:D].T
    Ws = msg_W[:, D : 2 * D].T
    # torch gate order in W_ih/W_hh: rows [r, z, n]
    Wir, Wiz, Win = W_ih[0:D], W_ih[D : 2 * D], W_ih[2 * D : 3 * D]
    Whr, Whz, Whn = W_hh[0:D], W_hh[D : 2 * D], W_hh[2 * D : 3 * D]

    mats = [
        Wx, Ws,
        -0.5 * Wiz.T, -0.5 * Whz.T,
        0.5 * Wir.T, 0.5 * Whr.T,
        Win.T, 0.5 * Whn.T,
        np.eye(D, dtype=np.float32),
    ]
    wblob = np.concatenate([_blkdiag(m) for m in mats], axis=1)

    bblob = np.concatenate([
        np.tile(-0.5 * (b_ih[D : 2 * D] + b_hh[D : 2 * D]), 2),
        np.tile(0.5 * (b_ih[0:D] + b_hh[0:D]), 2),
        np.tile(b_ih[2 * D : 3 * D], 2),
        np.tile(0.5 * b_hh[2 * D : 3 * D], 2),
    ]).reshape(1, 4 * 2 * D)

    shared = {
        "tb": tb_host,
        "wblob": wblob.astype(BF16),
        "bblob": bblob.astype(BF16),
    }
    in_maps = []
    for c in range(NCORES):
        m = dict(shared)
        m["xP"] = xP8[c]
        m["s0"] = s08[c]
        in_maps.append(m)
    return in_maps


def kernel(**inputs):
    from concourse.bass_utils import run_bass_kernel_spmd

    in_maps = _prep_host(**inputs)

    if "prog" not in _PROGRAM_CACHE:
        _PROGRAM_CACHE["prog"] = _build_program()
    nc = _PROGRAM_CACHE["prog"]

    res = run_bass_kernel_spmd(nc, in_maps, core_ids=list(range(NCORES)))
    _PROGRAM_CACHE["last_results"] = res

    out = np.empty((T, B * C, D), dtype=np.float32)
    for c in range(NCORES):
        outP_c = np.asarray(res.results[c]["outP"], dtype=np.float32)  # [T,128,256]
        base = c * ROWS
        out[:, base : base + HALF, :] = outP_c[:, 0:D, :].transpose(0, 2, 1)
        out[:, base + HALF : base + ROWS, :] = outP_c[:, D:, :].transpose(0, 2, 1)
    return out.reshape(T, B, C, D)


# revision 8
# speedup vs baseline: 1.2358x; 1.0272x over previous
"""Trainium2 Bass kernel for nn_MemoryNetwork (GRU-style memory network scan).

Model (per reference):
  t_enc = cos(arange(T) * freq + phase)                    [T, D]
  s0 = mean_t(x)                                           [B*C, D]
  tr = arange(T) * mask; x_seq = x[tr]; te_seq = t_enc[tr]
  per step t:
    msg = gelu([x_t, s, te_t] @ msg_W.T + msg_b)
    gi = msg @ W_ih.T + b_ih ; gh = s @ W_hh.T + b_hh
    r = sigmoid(i_r + h_r); z = sigmoid(i_z + h_z)
    n = tanh(i_n + r * h_n)
    s' = (1 - z) * n + z * s
  output: states [T, B, C, D]

Strategy: data-parallel over B*C = 4096 rows -> 8 cores x 512 rows.
On-device layout is feature-major and PARTITION-PACKED: two rows share a
packed column (row c on partitions 0:64, row 256+c on partitions 64:128),
with block-diagonal doubled weights [128, 128].  Matmul cost depends only
on the free dim, so packing halves the activation-engine work (the
bottleneck) for free.  The 256 packed columns split into NPB=3
independent chains (86/85/85).

Engines execute their queues IN PROGRAM ORDER, so the emission order is
SOFTWARE-PIPELINED: chain p runs phase (slot - p) % 3 of its step at each
slot, staggering the three chains across the three ACT ops of a step
(gelu / zr-tanh / n-tanh).  Between two consecutive ACT ops of one chain
the ACT engine executes the other two chains' (independent) ops, hiding
each chain's PE/DVE dependency latency.  Phases:

  A(t): s-dependent matmuls (Ws@s into pm; Whz/Whr@s + z/r bias rows into
        zr; Whn@s + bias into hh; b_in into pw), then u = gelu(pm).
  B(t): u-side matmuls (Wiz/Wir@u, Win@u), hzr = tanh([-a_z/2 | +a_r/2]),
        q = (hr+1)*hh on DVE, I@q accumulated into pw (PE),
        Q = 0.5*hz+0.5, Q' = -0.5*hz+0.5 (= 1-z, z), w1 = Q'*s on DVE.
  C(t): n = tanh(pw), v = Q*n, s' = v + w1 (= (1-z)*n + z*s), Pool-copy
        s' to the output stage, and prehoist step t+1's x-dependent
        matmuls (tb broadcast row with start=True, Wx@x) into a fresh
        PSUM bank.

All gate scales/biases are folded into the doubled weights or added in
PSUM via broadcast-row matmuls, so every ACT op is plain func(x).  Each
chain-step owns one PSUM bank [128, 512] f32 (one start=True / one
stop=True per bank: start zeroes the whole 2KB zero-region).  Outputs
stream out as bf16 every CH steps; the host unpacks partitions to rows.
"""

import sys

import numpy as np

sys.path.insert(0, "/opt/trn_rl_repo")

import ml_dtypes  # noqa: E402

BF16 = ml_dtypes.bfloat16

T, B, C, D = 256, 64, 64, 64
NCORES = 8
ROWS = (B * C) // NCORES  # 512 rows per core
HALF = ROWS // 2  # 256 packed columns (2 rows per column)
CH = 8  # timesteps per DMA chunk
NPB = 3  # packed chains per core
PF = [86, 85, 85]  # packed columns per chain
POFF = [0, 86, 171, 256]

_PROGRAM_CACHE = {}


def _build_program():
    import concourse.bacc as bacc
    import concourse.tile as tile
    from concourse import mybir
    from contextlib import ExitStack

    BF = mybir.dt.bfloat16
    F32 = mybir.dt.float32
    AF = mybir.ActivationFunctionType
    OP = mybir.AluOpType

    nc = bacc.Bacc(None, target_bir_lowering=False, debug=False)

    xP = nc.dram_tensor("xP", [T, 2 * D, HALF], BF, kind="ExternalInput")
    s0 = nc.dram_tensor("s0", [2 * D, HALF], BF, kind="ExternalInput")
    tb = nc.dram_tensor("tb", [1, T, 2 * D], BF, kind="ExternalInput")
    # 8 block-diagonal doubled weight mats + identity, packed [128, 9*128]:
    #   wx, ws, wiz(-.5), whz(-.5), wir(+.5), whr(+.5), win, whn(.5), I
    wblob = nc.dram_tensor("wblob", [2 * D, 9 * 2 * D], BF, kind="ExternalInput")
    # bias rows on partition 0, side by side [1, 4*128]:
    # -0.5(b_iz+b_hz) | +0.5(b_ir+b_hr) | b_in | 0.5*b_hn  (each tiled 2x)
    bblob = nc.dram_tensor("bblob", [1, 4 * 2 * D], BF, kind="ExternalInput")
    outP = nc.dram_tensor("outP", [T, 2 * D, HALF], BF, kind="ExternalOutput")

    DD = 2 * D  # 128

    with ExitStack() as ctx:
        tc = ctx.enter_context(tile.TileContext(nc))
        consts = ctx.enter_context(tc.tile_pool(name="consts", bufs=1))
        xpool = ctx.enter_context(tc.tile_pool(name="xc", bufs=2))
        opool = ctx.enter_context(tc.tile_pool(name="ostage", bufs=2))
        spool = ctx.enter_context(tc.tile_pool(name="state", bufs=3))
        upool = ctx.enter_context(tc.tile_pool(name="u", bufs=2))
        gpool = ctx.enter_context(tc.tile_pool(name="gates", bufs=2))
        psum = ctx.enter_context(tc.tile_pool(name="psum", bufs=2, space="PSUM"))

        wblob_sb = consts.tile([DD, 9 * DD], BF, tag="wblob")
        nc.sync.dma_start(out=wblob_sb, in_=wblob[:, :])
        bblob_sb = consts.tile([1, 4 * DD], BF, tag="bblob")
        nc.sync.dma_start(out=bblob_sb, in_=bblob[:, :])
        tb_sb = consts.tile([1, T, DD], BF, tag="tb")
        nc.sync.dma_start(out=tb_sb, in_=tb[:, :, :])

        wx_sb = wblob_sb[:, 0 * DD : 1 * DD]
        ws_sb = wblob_sb[:, 1 * DD : 2 * DD]
        wiz_sb = wblob_sb[:, 2 * DD : 3 * DD]
        whz_sb = wblob_sb[:, 3 * DD : 4 * DD]
        wir_sb = wblob_sb[:, 4 * DD : 5 * DD]
        whr_sb = wblob_sb[:, 5 * DD : 6 * DD]
        win_sb = wblob_sb[:, 6 * DD : 7 * DD]
        whn_sb = wblob_sb[:, 7 * DD : 8 * DD]
        ident_sb = wblob_sb[:, 8 * DD : 9 * DD]
        bz_sb = bblob_sb[0:1, 0 * DD : 1 * DD]
        br_sb = bblob_sb[0:1, 1 * DD : 2 * DD]
        bin_sb = bblob_sb[0:1, 2 * DD : 3 * DD]
        bhn_sb = bblob_sb[0:1, 3 * DD : 4 * DD]
        ones_sb = consts.tile([1, HALF], BF)
        nc.vector.memset(ones_sb, 1.0)

        # --- pipeline state per chain ---
        R = []
        for p in range(NPB):
            st = spool.tile([DD, PF[p]], BF, tag=f"state{p}")
            nc.sync.dma_start(out=st, in_=s0[:, POFF[p] : POFF[p + 1]])
            R.append({"s": st})

        chunks = {}  # chunk idx -> xc tile
        ostages = {}  # chunk idx -> ostage tile

        def get_chunk(c):
            if c not in chunks:
                t0 = c * CH
                xc = xpool.tile([DD, CH, HALF], BF, tag="xc", name="xc")
                nc.sync.dma_start(
                    out=xc, in_=xP[t0 : t0 + CH, :, :].rearrange("c p r -> p c r")
                )
                chunks[c] = xc
            return chunks[c]

        def get_ostage(c):
            if c not in ostages:
                ostages[c] = opool.tile([DD, CH, HALF], BF, tag="ostage", name="ostage")
            return ostages[c]

        def regions(bank, F):
            return (bank[:, 0:F], bank[:, F : 3 * F], bank[:, F : 2 * F],
                    bank[:, 2 * F : 3 * F], bank[:, 3 * F : 4 * F],
                    bank[:, 4 * F : 5 * F])

        def prehoist(p, t):
            """Fresh PSUM bank for (p, t): tb broadcast (start=True, zeroes
            the bank) + Wx@x.  Emitted one phase before A(t)."""
            F = PF[p]
            cs = slice(POFF[p], POFF[p + 1])
            bank = psum.tile([DD, 512], F32, tag=f"bank{p}", name=f"bank{p}")
            pm = bank[:, 0:F]
            nc.tensor.matmul(
                pm, tb_sb[:, t, :], ones_sb[:, 0:F], start=True, stop=False
            )
            xc = get_chunk(t // CH)
            nc.tensor.matmul(pm, wx_sb, xc[:, t % CH, cs], start=False, stop=False)
            R[p]["bank"] = bank

        def phaseA(p, t):
            F = PF[p]
            s_b = R[p]["s"]
            R[p]["bank_cur"] = R[p]["bank"]
            bank = R[p]["bank_cur"]
            pm, zr, zz, rr, pw, hh = regions(bank, F)
            if t == 0:
                nc.tensor.matmul(pm, ws_sb, s_b, start=False, stop=False)
            nc.tensor.matmul(zz, bz_sb, ones_sb[:, 0:F], start=False, stop=False)
            nc.tensor.matmul(rr, br_sb, ones_sb[:, 0:F], start=False, stop=False)
            nc.tensor.matmul(zz, whz_sb, s_b, start=False, stop=False)
            nc.tensor.matmul(rr, whr_sb, s_b, start=False, stop=False)
            nc.tensor.matmul(hh, bhn_sb, ones_sb[:, 0:F], start=False, stop=False)
            nc.tensor.matmul(hh, whn_sb, s_b, start=False, stop=False)
            nc.tensor.matmul(pw, bin_sb, ones_sb[:, 0:F], start=False, stop=False)
            u = upool.tile([DD, F], BF, tag=f"u{p}", name=f"u{p}")
            nc.scalar.activation(u, pm, AF.Gelu)
            R[p]["u"] = u

        def phaseB(p, t):
            F = PF[p]
            s_b = R[p]["s"]
            bank = R[p]["bank_cur"]
            pm, zr, zz, rr, pw, hh = regions(bank, F)
            u = R[p]["u"]
            nc.tensor.matmul(zz, wiz_sb, u, start=False, stop=False)
            nc.tensor.matmul(rr, wir_sb, u, start=False, stop=False)
            nc.tensor.matmul(pw, win_sb, u, start=False, stop=False)
            hzr = gpool.tile([DD, 2 * F], BF, tag=f"hzr{p}", name=f"hzr{p}")
            nc.scalar.activation(hzr, zr, AF.Tanh)
            hz = hzr[:, 0:F]
            hr = hzr[:, F : 2 * F]
            # q = (hr + 1) * hh  (DVE; GPSIMD cannot access PSUM)
            q = gpool.tile([DD, F], BF, tag=f"q{p}", name=f"q{p}")
            nc.vector.scalar_tensor_tensor(q, hr, 1.0, hh, OP.add, OP.mult)
            nc.tensor.matmul(pw, ident_sb, q, start=False, stop=True)
            # blend coefficients: Q = 1-z, Q' = z;  w1 = z*s
            Q = gpool.tile([DD, F], BF, tag=f"Q{p}", name=f"Qc{p}")
            nc.vector.tensor_scalar(Q, hz, 0.5, 0.5, OP.mult, OP.add)
            Qp = gpool.tile([DD, F], BF, tag=f"Qp{p}", name=f"Qp{p}")
            nc.vector.tensor_scalar(Qp, hz, -0.5, 0.5, OP.mult, OP.add)
            w1 = gpool.tile([DD, F], BF, tag=f"w1{p}", name=f"w1{p}")
            nc.vector.tensor_mul(w1, Qp, s_b)
            R[p]["Q"] = Q
            R[p]["w1"] = w1
            # prehoist step t+1's x-side matmuls, then Ws@w1 (Ws@s' is
            # split linearly: Ws@s' = Ws@v + Ws@w1, so the gelu for t+1
            # never waits on s' itself)
            if t + 1 < T:
                prehoist(p, t + 1)
                nbank = R[p]["bank"]
                nc.tensor.matmul(
                    nbank[:, 0:F], ws_sb, w1, start=False, stop=False
                )

        def phaseC(p, t):
            F = PF[p]
            cs = slice(POFF[p], POFF[p + 1])
            bank = R[p]["bank_cur"]
            pw = bank[:, 3 * F : 4 * F]
            n = gpool.tile([DD, F], BF, tag=f"n{p}", name=f"n{p}")
            nc.scalar.activation(n, pw, AF.Tanh)
            v = gpool.tile([DD, F], BF, tag=f"v{p}", name=f"v{p}")
            nc.vector.tensor_mul(v, R[p]["Q"], n)
            if t + 1 < T:
                nbank = R[p]["bank"]
                nc.tensor.matmul(nbank[:, 0:F], ws_sb, v, start=False, stop=False)
            s_nxt = spool.tile([DD, F], BF, tag=f"state{p}", name=f"state{p}")
            nc.vector.tensor_add(s_nxt, v, R[p]["w1"])
            ost = get_ostage(t // CH)
            nc.gpsimd.tensor_copy(out=ost[:, t % CH, cs], in_=s_nxt)
            R[p]["s"] = s_nxt
            if p == NPB - 1 and t % CH == CH - 1:
                c = t // CH
                nc.sync.dma_start(
                    out=outP[t - CH + 1 : t + 1, :, :].rearrange("c p r -> p c r"),
                    in_=ostages.pop(c),
                )
                chunks.pop(c, None)

        # --- bootstrap: bank(0) + x-mms for every chain ---
        for p in range(NPB):
            prehoist(p, 0)

        # --- pipelined slot loop: chain p does phase (i - p) % 3 ---
        PHASES = [phaseA, phaseB, phaseC]
        for i in range(3 * T + 2):
            for p in range(NPB):
                ph = (i - p) % 3
                t = (i - p) // 3
                if 0 <= t < T:
                    PHASES[ph](p, t)

    nc.compile()
    return nc


def _blkdiag(a):
    """[64, 64] -> [128, 128] block-diagonal double."""
    out = np.zeros((2 * D, 2 * D), np.float32)
    out[:D, :D] = a
    out[D:, D:] = a
    return out


def _prep_host(x, mask, msg_W, msg_b, W_ih, W_hh, b_ih, b_hh, basis_freq, phase):
    """Host-side prep: partition-packing, sharding, weight doubling."""
    x = np.asarray(x, dtype=np.float32)
    mask = np.asarray(mask)
    msg_W = np.asarray(msg_W, np.float32)
    msg_b = np.asarray(msg_b, np.float32)
    W_ih = np.asarray(W_ih, np.float32)
    W_hh = np.asarray(W_hh, np.float32)
    b_ih = np.asarray(b_ih, np.float32)
    b_hh = np.asarray(b_hh, np.float32)
    basis_freq = np.asarray(basis_freq, np.float32)
    phase = np.asarray(phase, np.float32)

    tr = np.arange(T, dtype=np.int64) * mask.astype(np.int64)
    identity_gather = bool(np.array_equal(tr, np.arange(T)))

    xf = x.reshape(T, B * C, D)
    s0_rows = xf.mean(axis=0)  # [B*C, D] f32 (from ungathered x)
    if not identity_gather:
        xf = xf[tr]

    xP8, s08 = [], []
    for c in range(NCORES):
        blk = xf[:, c * ROWS : (c + 1) * ROWS, :]  # [T, 512, 64]
        lo = blk[:, 0:HALF].transpose(0, 2, 1)  # [T, 64, 256]
        hi = blk[:, HALF:ROWS].transpose(0, 2, 1)
        xP8.append(np.ascontiguousarray(
            np.concatenate([lo, hi], axis=1)).astype(BF16))
        sblk = s0_rows[c * ROWS : (c + 1) * ROWS]  # [512, 64]
        s08.append(np.ascontiguousarray(np.concatenate(
            [sblk[0:HALF].T, sblk[HALF:ROWS].T], axis=0)).astype(BF16))

    ts_ = np.arange(T, dtype=np.float32)[tr]
    te = np.cos(ts_[:, None] * basis_freq[None, :] + phase[None, :])  # [T, D]
    Wt = msg_W[:, 2 * D : 3 * D]
    tb1 = te @ Wt.T + msg_b[None, :]  # [T, 64]
    tb_host = np.tile(tb1, (1, 2)).astype(BF16).reshape(1, T, 2 * D)

    Wx = msg_W[:, 0:D].T
    Ws = msg_W[:, D : 2 * D].T
    # torch gate order in W_ih/W_hh: rows [r, z, n]
    Wir, Wiz, Win = W_ih[0:D], W_ih[D : 2 * D], W_ih[2 * D : 3 * D]
    Whr, Whz, Whn = W_hh[0:D], W_hh[D : 2 * D], W_hh[2 * D : 3 * D]

    mats = [
        Wx, Ws,
        -0.5 * Wiz.T, -0.5 * Whz.T,
        0.5 * Wir.T, 0.5 * Whr.T,
        Win.T, 0.5 * Whn.T,
        np.eye(D, dtype=np.float32),
    ]
    wblob = np.concatenate([_blkdiag(m) for m in mats], axis=1)

    bblob = np.concatenate([
        np.tile(-0.5 * (b_ih[D : 2 * D] + b_hh[D : 2 * D]), 2),
        np.tile(0.5 * (b_ih[0:D] + b_hh[0:D]), 2),
        np.tile(b_ih[2 * D : 3 * D], 2),
        np.tile(0.5 * b_hh[2 * D : 3 * D], 2),
    ]).reshape(1, 4 * 2 * D)

    shared = {
        "tb": tb_host,
        "wblob": wblob.astype(BF16),
        "bblob": bblob.astype(BF16),
    }
    in_maps = []
    for c in range(NCORES):
        m = dict(shared)
        m["xP"] = xP8[c]
        m["s0"] = s08[c]
        in_maps.append(m)
    return in_maps


def kernel(**inputs):
    from concourse.bass_utils import run_bass_kernel_spmd

    in_maps = _prep_host(**inputs)

    if "prog" not in _PROGRAM_CACHE:
        _PROGRAM_CACHE["prog"] = _build_program()
    nc = _PROGRAM_CACHE["prog"]

    res = run_bass_kernel_spmd(nc, in_maps, core_ids=list(range(NCORES)))
    _PROGRAM_CACHE["last_results"] = res

    out = np.empty((T, B * C, D), dtype=np.float32)
    for c in range(NCORES):
        outP_c = np.asarray(res.results[c]["outP"], dtype=np.float32)  # [T,128,256]
        base = c * ROWS
        out[:, base : base + HALF, :] = outP_c[:, 0:D, :].transpose(0, 2, 1)
        out[:, base + HALF : base + ROWS, :] = outP_c[:, D:, :].transpose(0, 2, 1)
    return out.reshape(T, B, C, D)


# revision 9
# speedup vs baseline: 1.2624x; 1.0216x over previous
"""Trainium2 Bass kernel for nn_MemoryNetwork (GRU-style memory network scan).

Model (per reference):
  t_enc = cos(arange(T) * freq + phase)                    [T, D]
  s0 = mean_t(x)                                           [B*C, D]
  tr = arange(T) * mask; x_seq = x[tr]; te_seq = t_enc[tr]
  per step t:
    msg = gelu([x_t, s, te_t] @ msg_W.T + msg_b)
    gi = msg @ W_ih.T + b_ih ; gh = s @ W_hh.T + b_hh
    r = sigmoid(i_r + h_r); z = sigmoid(i_z + h_z)
    n = tanh(i_n + r * h_n)
    s' = (1 - z) * n + z * s
  output: states [T, B, C, D]

Strategy: data-parallel over B*C = 4096 rows -> 8 cores x 512 rows.
On-device layout is feature-major and PARTITION-PACKED: two rows share a
packed column (row c on partitions 0:64, row 256+c on partitions 64:128),
with block-diagonal doubled weights [128, 128].  Matmul cost depends only
on the free dim, so packing halves the activation-engine work (the
bottleneck) for free.  The 256 packed columns split into NPB=3
independent chains (86/85/85).

Engines execute their queues IN PROGRAM ORDER, so the emission order is
SOFTWARE-PIPELINED: chain p runs phase (slot - p) % 3 of its step at each
slot, staggering the three chains across the three ACT ops of a step
(gelu / zr-tanh / n-tanh).  Between two consecutive ACT ops of one chain
the ACT engine executes the other two chains' (independent) ops, hiding
each chain's PE/DVE dependency latency.  Phases:

  A(t): s-dependent matmuls (Ws@s into pm; Whz/Whr@s + z/r bias rows into
        zr; Whn@s + bias into hh; b_in into pw), then u = gelu(pm).
  B(t): u-side matmuls (Wiz/Wir@u, Win@u), hzr = tanh([-a_z/2 | +a_r/2]),
        q = (hr+1)*hh on DVE, I@q accumulated into pw (PE),
        Q = 0.5*hz+0.5, Q' = -0.5*hz+0.5 (= 1-z, z), w1 = Q'*s on DVE.
  C(t): n = tanh(pw), v = Q*n, s' = v + w1 (= (1-z)*n + z*s), Pool-copy
        s' to the output stage, and prehoist step t+1's x-dependent
        matmuls (tb broadcast row with start=True, Wx@x) into a fresh
        PSUM bank.

All gate scales/biases are folded into the doubled weights or added in
PSUM via broadcast-row matmuls, so every ACT op is plain func(x).  Each
chain-step owns one PSUM bank [128, 512] f32 (one start=True / one
stop=True per bank: start zeroes the whole 2KB zero-region).  Outputs
stream out as bf16 every CH steps; the host unpacks partitions to rows.
"""

import sys

import numpy as np

sys.path.insert(0, "/opt/trn_rl_repo")

import ml_dtypes  # noqa: E402

BF16 = ml_dtypes.bfloat16

T, B, C, D = 256, 64, 64, 64
NCORES = 8
ROWS = (B * C) // NCORES  # 512 rows per core
HALF = ROWS // 2  # 256 packed columns (2 rows per column)
CH = 8  # timesteps per DMA chunk
NPB = 3  # packed chains per core
PF = [86, 85, 85]  # packed columns per chain
POFF = [0, 86, 171, 256]

_PROGRAM_CACHE = {}


def _build_program():
    import concourse.bacc as bacc
    import concourse.tile as tile
    from concourse import mybir
    from contextlib import ExitStack

    BF = mybir.dt.bfloat16
    F32 = mybir.dt.float32
    AF = mybir.ActivationFunctionType
    OP = mybir.AluOpType

    nc = bacc.Bacc(None, target_bir_lowering=False, debug=False)

    xP = nc.dram_tensor("xP", [T, 2 * D, HALF], BF, kind="ExternalInput")
    s0 = nc.dram_tensor("s0", [2 * D, HALF], BF, kind="ExternalInput")
    tb = nc.dram_tensor("tb", [1, T, 2 * D], BF, kind="ExternalInput")
    # 8 block-diagonal doubled weight mats + identity, packed [128, 9*128]:
    #   wx, ws, wiz(-.5), whz(-.5), wir(+.5), whr(+.5), win, whn(.5), I
    wblob = nc.dram_tensor("wblob", [2 * D, 9 * 2 * D], BF, kind="ExternalInput")
    # bias rows on partition 0, side by side [1, 4*128]:
    # -0.5(b_iz+b_hz) | +0.5(b_ir+b_hr) | b_in | 0.5*b_hn  (each tiled 2x)
    bblob = nc.dram_tensor("bblob", [1, 4 * 2 * D], BF, kind="ExternalInput")
    outP = nc.dram_tensor("outP", [T, 2 * D, HALF], BF, kind="ExternalOutput")

    DD = 2 * D  # 128

    with ExitStack() as ctx:
        tc = ctx.enter_context(tile.TileContext(nc))
        consts = ctx.enter_context(tc.tile_pool(name="consts", bufs=1))
        xpool = ctx.enter_context(tc.tile_pool(name="xc", bufs=2))
        opool = ctx.enter_context(tc.tile_pool(name="ostage", bufs=2))
        spool = ctx.enter_context(tc.tile_pool(name="state", bufs=3))
        upool = ctx.enter_context(tc.tile_pool(name="u", bufs=2))
        gpool = ctx.enter_context(tc.tile_pool(name="gates", bufs=2))
        psum = ctx.enter_context(tc.tile_pool(name="psum", bufs=2, space="PSUM"))

        wblob_sb = consts.tile([DD, 9 * DD], BF, tag="wblob")
        nc.sync.dma_start(out=wblob_sb, in_=wblob[:, :])
        bblob_sb = consts.tile([1, 4 * DD], BF, tag="bblob")
        nc.sync.dma_start(out=bblob_sb, in_=bblob[:, :])
        tb_sb = consts.tile([1, T, DD], BF, tag="tb")
        for g in range(4):
            g0 = g * (T // 4)
            nc.sync.dma_start(
                out=tb_sb[:, g0 : g0 + T // 4, :],
                in_=tb[:, g0 : g0 + T // 4, :],
            )

        wx_sb = wblob_sb[:, 0 * DD : 1 * DD]
        ws_sb = wblob_sb[:, 1 * DD : 2 * DD]
        wiz_sb = wblob_sb[:, 2 * DD : 3 * DD]
        whz_sb = wblob_sb[:, 3 * DD : 4 * DD]
        wir_sb = wblob_sb[:, 4 * DD : 5 * DD]
        whr_sb = wblob_sb[:, 5 * DD : 6 * DD]
        win_sb = wblob_sb[:, 6 * DD : 7 * DD]
        whn_sb = wblob_sb[:, 7 * DD : 8 * DD]
        ident_sb = wblob_sb[:, 8 * DD : 9 * DD]
        bz_sb = bblob_sb[0:1, 0 * DD : 1 * DD]
        br_sb = bblob_sb[0:1, 1 * DD : 2 * DD]
        bin_sb = bblob_sb[0:1, 2 * DD : 3 * DD]
        bhn_sb = bblob_sb[0:1, 3 * DD : 4 * DD]
        ones_sb = consts.tile([1, HALF], BF)
        nc.vector.memset(ones_sb, 1.0)

        # --- pipeline state per chain ---
        R = []
        for p in range(NPB):
            st = spool.tile([DD, PF[p]], BF, tag=f"state{p}")
            nc.sync.dma_start(out=st, in_=s0[:, POFF[p] : POFF[p + 1]])
            R.append({"s": st})

        chunks = {}  # chunk idx -> xc tile
        ostages = {}  # chunk idx -> ostage tile

        def get_chunk(c):
            if c not in chunks:
                t0 = c * CH
                xc = xpool.tile([DD, CH, HALF], BF, tag="xc", name="xc")
                nc.sync.dma_start(
                    out=xc, in_=xP[t0 : t0 + CH, :, :].rearrange("c p r -> p c r")
                )
                chunks[c] = xc
            return chunks[c]

        def get_ostage(c):
            if c not in ostages:
                ostages[c] = opool.tile([DD, CH, HALF], BF, tag="ostage", name="ostage")
            return ostages[c]

        def regions(bank, F):
            return (bank[:, 0:F], bank[:, F : 3 * F], bank[:, F : 2 * F],
                    bank[:, 2 * F : 3 * F], bank[:, 3 * F : 4 * F],
                    bank[:, 4 * F : 5 * F])

        def prehoist(p, t):
            """Fresh PSUM bank for (p, t): tb broadcast (start=True, zeroes
            the bank) + Wx@x.  Emitted one phase before A(t)."""
            F = PF[p]
            cs = slice(POFF[p], POFF[p + 1])
            bank = psum.tile([DD, 512], F32, tag=f"bank{p}", name=f"bank{p}")
            pm = bank[:, 0:F]
            nc.tensor.matmul(
                pm, tb_sb[:, t, :], ones_sb[:, 0:F], start=True, stop=False
            )
            xc = get_chunk(t // CH)
            nc.tensor.matmul(pm, wx_sb, xc[:, t % CH, cs], start=False, stop=False)
            R[p]["bank"] = bank

        def phaseA(p, t):
            F = PF[p]
            s_b = R[p]["s"]
            R[p]["bank_cur"] = R[p]["bank"]
            bank = R[p]["bank_cur"]
            pm, zr, zz, rr, pw, hh = regions(bank, F)
            if t == 0:
                nc.tensor.matmul(pm, ws_sb, s_b, start=False, stop=False)
            nc.tensor.matmul(zz, bz_sb, ones_sb[:, 0:F], start=False, stop=False)
            nc.tensor.matmul(rr, br_sb, ones_sb[:, 0:F], start=False, stop=False)
            nc.tensor.matmul(zz, whz_sb, s_b, start=False, stop=False)
            nc.tensor.matmul(rr, whr_sb, s_b, start=False, stop=False)
            nc.tensor.matmul(hh, bhn_sb, ones_sb[:, 0:F], start=False, stop=False)
            nc.tensor.matmul(hh, whn_sb, s_b, start=False, stop=False)
            nc.tensor.matmul(pw, bin_sb, ones_sb[:, 0:F], start=False, stop=False)
            u = upool.tile([DD, F], BF, tag=f"u{p}", name=f"u{p}")
            nc.scalar.activation(u, pm, AF.Gelu)
            R[p]["u"] = u

        def phaseB(p, t):
            F = PF[p]
            s_b = R[p]["s"]
            bank = R[p]["bank_cur"]
            pm, zr, zz, rr, pw, hh = regions(bank, F)
            u = R[p]["u"]
            nc.tensor.matmul(zz, wiz_sb, u, start=False, stop=False)
            nc.tensor.matmul(rr, wir_sb, u, start=False, stop=False)
            nc.tensor.matmul(pw, win_sb, u, start=False, stop=False)
            hzr = gpool.tile([DD, 2 * F], BF, tag=f"hzr{p}", name=f"hzr{p}")
            nc.scalar.activation(hzr, zr, AF.Tanh)
            hz = hzr[:, 0:F]
            hr = hzr[:, F : 2 * F]
            # q = (hr + 1) * hh  (DVE; GPSIMD cannot access PSUM)
            q = gpool.tile([DD, F], BF, tag=f"q{p}", name=f"q{p}")
            nc.vector.scalar_tensor_tensor(q, hr, 1.0, hh, OP.add, OP.mult)
            nc.tensor.matmul(pw, ident_sb, q, start=False, stop=True)
            # blend coefficients: Q = 1-z, Q' = z;  w1 = z*s
            Q = gpool.tile([DD, F], BF, tag=f"Q{p}", name=f"Qc{p}")
            nc.vector.tensor_scalar(Q, hz, 0.5, 0.5, OP.mult, OP.add)
            Qp = gpool.tile([DD, F], BF, tag=f"Qp{p}", name=f"Qp{p}")
            nc.vector.tensor_scalar(Qp, hz, -0.5, 0.5, OP.mult, OP.add)
            w1 = gpool.tile([DD, F], BF, tag=f"w1{p}", name=f"w1{p}")
            nc.vector.tensor_mul(w1, Qp, s_b)
            R[p]["Q"] = Q
            R[p]["w1"] = w1
            # prehoist step t+1's x-side matmuls, then Ws@w1 (Ws@s' is
            # split linearly: Ws@s' = Ws@v + Ws@w1, so the gelu for t+1
            # never waits on s' itself)
            if t + 1 < T:
                prehoist(p, t + 1)
                nbank = R[p]["bank"]
                nc.tensor.matmul(
                    nbank[:, 0:F], ws_sb, w1, start=False, stop=False
                )

        def phaseC(p, t):
            F = PF[p]
            bank = R[p]["bank_cur"]
            pw = bank[:, 3 * F : 4 * F]
            n = gpool.tile([DD, F], BF, tag=f"n{p}", name=f"n{p}")
            nc.scalar.activation(n, pw, AF.Tanh)
            R[p]["n"] = n

        def phaseD(p, t):
            F = PF[p]
            cs = slice(POFF[p], POFF[p + 1])
            n = R[p]["n"]
            v = gpool.tile([DD, F], BF, tag=f"v{p}", name=f"v{p}")
            nc.vector.tensor_mul(v, R[p]["Q"], n)
            if t + 1 < T:
                nbank = R[p]["bank"]
                nc.tensor.matmul(nbank[:, 0:F], ws_sb, v, start=False, stop=False)
            s_nxt = spool.tile([DD, F], BF, tag=f"state{p}", name=f"state{p}")
            nc.vector.tensor_add(s_nxt, v, R[p]["w1"])
            ost = get_ostage(t // CH)
            nc.gpsimd.tensor_copy(out=ost[:, t % CH, cs], in_=s_nxt)
            R[p]["s"] = s_nxt
            if p == NPB - 1 and t % CH == CH - 1:
                c = t // CH
                nc.sync.dma_start(
                    out=outP[t - CH + 1 : t + 1, :, :].rearrange("c p r -> p c r"),
                    in_=ostages.pop(c),
                )
                chunks.pop(c, None)

        # --- bootstrap: bank(0) + x-mms for every chain ---
        for p in range(NPB):
            prehoist(p, 0)

        # --- pipelined slot loop: chain p does phase (i - p) % 4 ---
        PHASES = [phaseA, phaseB, phaseC, phaseD]
        for i in range(4 * T + 3):
            for p in range(NPB):
                ph = (i - p) % 4
                t = (i - p) // 4
                if 0 <= t < T:
                    PHASES[ph](p, t)

    nc.compile()
    return nc


def _blkdiag(a):
    """[64, 64] -> [128, 128] block-diagonal double."""
    out = np.zeros((2 * D, 2 * D), np.float32)
    out[:D, :D] = a
    out[D:, D:] = a
    return out


def _prep_host(x, mask, msg_W, msg_b, W_ih, W_hh, b_ih, b_hh, basis_freq, phase):
    """Host-side prep: partition-packing, sharding, weight doubling."""
    x = np.asarray(x, dtype=np.float32)
    mask = np.asarray(mask)
    msg_W = np.asarray(msg_W, np.float32)
    msg_b = np.asarray(msg_b, np.float32)
    W_ih = np.asarray(W_ih, np.float32)
    W_hh = np.asarray(W_hh, np.float32)
    b_ih = np.asarray(b_ih, np.float32)
    b_hh = np.asarray(b_hh, np.float32)
    basis_freq = np.asarray(basis_freq, np.float32)
    phase = np.asarray(phase, np.float32)

    tr = np.arange(T, dtype=np.int64) * mask.astype(np.int64)
    identity_gather = bool(np.array_equal(tr, np.arange(T)))

    xf = x.reshape(T, B * C, D)
    s0_rows = xf.mean(axis=0)  # [B*C, D] f32 (from ungathered x)
    if not identity_gather:
        xf = xf[tr]

    xP8, s08 = [], []
    for c in range(NCORES):
        blk = xf[:, c * ROWS : (c + 1) * ROWS, :]  # [T, 512, 64]
        lo = blk[:, 0:HALF].transpose(0, 2, 1)  # [T, 64, 256]
        hi = blk[:, HALF:ROWS].transpose(0, 2, 1)
        xP8.append(np.ascontiguousarray(
            np.concatenate([lo, hi], axis=1)).astype(BF16))
        sblk = s0_rows[c * ROWS : (c + 1) * ROWS]  # [512, 64]
        s08.append(np.ascontiguousarray(np.concatenate(
            [sblk[0:HALF].T, sblk[HALF:ROWS].T], axis=0)).astype(BF16))

    ts_ = np.arange(T, dtype=np.float32)[tr]
    te = np.cos(ts_[:, None] * basis_freq[None, :] + phase[None, :])  # [T, D]
    Wt = msg_W[:, 2 * D : 3 * D]
    tb1 = te @ Wt.T + msg_b[None, :]  # [T, 64]
    tb_host = np.tile(tb1, (1, 2)).astype(BF16).reshape(1, T, 2 * D)

    Wx = msg_W[:, 0:D].T
    Ws = msg_W[:, D : 2 * D].T
    # torch gate order in W_ih/W_hh: rows [r, z, n]
    Wir, Wiz, Win = W_ih[0:D], W_ih[D : 2 * D], W_ih[2 * D : 3 * D]
    Whr, Whz, Whn = W_hh[0:D], W_hh[D : 2 * D], W_hh[2 * D : 3 * D]

    mats = [
        Wx, Ws,
        -0.5 * Wiz.T, -0.5 * Whz.T,
        0.5 * Wir.T, 0.5 * Whr.T,
        Win.T, 0.5 * Whn.T,
        np.eye(D, dtype=np.float32),
    ]
    wblob = np.concatenate([_blkdiag(m) for m in mats], axis=1)

    bblob = np.concatenate([
        np.tile(-0.5 * (b_ih[D : 2 * D] + b_hh[D : 2 * D]), 2),
        np.tile(0.5 * (b_ih[0:D] + b_hh[0:D]), 2),
        np.tile(b_ih[2 * D : 3 * D], 2),
        np.tile(0.5 * b_hh[2 * D : 3 * D], 2),
    ]).reshape(1, 4 * 2 * D)

    shared = {
        "tb": tb_host,
        "wblob": wblob.astype(BF16),
        "bblob": bblob.astype(BF16),
    }
    in_maps = []
    for c in range(NCORES):
        m = dict(shared)
        m["xP"] = xP8[c]
        m["s0"] = s08[c]
        in_maps.append(m)
    return in_maps


def kernel(**inputs):
    from concourse.bass_utils import run_bass_kernel_spmd

    in_maps = _prep_host(**inputs)

    if "prog" not in _PROGRAM_CACHE:
        _PROGRAM_CACHE["prog"] = _build_program()
    nc = _PROGRAM_CACHE["prog"]

    res = run_bass_kernel_spmd(nc, in_maps, core_ids=list(range(NCORES)))
    _PROGRAM_CACHE["last_results"] = res

    out = np.empty((T, B * C, D), dtype=np.float32)
    for c in range(NCORES):
        outP_c = np.asarray(res.results[c]["outP"], dtype=np.float32)  # [T,128,256]
        base = c * ROWS
        out[:, base : base + HALF, :] = outP_c[:, 0:D, :].transpose(0, 2, 1)
        out[:, base + HALF : base + ROWS, :] = outP_c[:, D:, :].transpose(0, 2, 1)
    return out.reshape(T, B, C, D)


# revision 10
# speedup vs baseline: 1.3036x; 1.0326x over previous
"""Trainium2 Bass kernel for nn_MemoryNetwork (GRU-style memory network scan).

Model (per reference):
  t_enc = cos(arange(T) * freq + phase)                    [T, D]
  s0 = mean_t(x)                                           [B*C, D]
  tr = arange(T) * mask; x_seq = x[tr]; te_seq = t_enc[tr]
  per step t:
    msg = gelu([x_t, s, te_t] @ msg_W.T + msg_b)
    gi = msg @ W_ih.T + b_ih ; gh = s @ W_hh.T + b_hh
    r = sigmoid(i_r + h_r); z = sigmoid(i_z + h_z)
    n = tanh(i_n + r * h_n)
    s' = (1 - z) * n + z * s
  output: states [T, B, C, D]

Strategy: data-parallel over B*C = 4096 rows -> 8 cores x 512 rows.
On-device layout is feature-major and PARTITION-PACKED: two rows share a
packed column (row c on partitions 0:64, row 256+c on partitions 64:128),
with block-diagonal doubled weights [128, 128].  Matmul cost depends only
on the free dim, so packing halves the activation-engine work (the
bottleneck) for free.  The 256 packed columns split into NPB=3
independent chains (86/85/85).

Engines execute their queues IN PROGRAM ORDER, so the emission order is
SOFTWARE-PIPELINED: chain p runs phase (slot - p) % 3 of its step at each
slot, staggering the three chains across the three ACT ops of a step
(gelu / zr-tanh / n-tanh).  Between two consecutive ACT ops of one chain
the ACT engine executes the other two chains' (independent) ops, hiding
each chain's PE/DVE dependency latency.  Phases:

  A(t): s-dependent matmuls (Ws@s into pm; Whz/Whr@s + z/r bias rows into
        zr; Whn@s + bias into hh; b_in into pw), then u = gelu(pm).
  B(t): u-side matmuls (Wiz/Wir@u, Win@u), hzr = tanh([-a_z/2 | +a_r/2]),
        q = (hr+1)*hh on DVE, I@q accumulated into pw (PE),
        Q = 0.5*hz+0.5, Q' = -0.5*hz+0.5 (= 1-z, z), w1 = Q'*s on DVE.
  C(t): n = tanh(pw), v = Q*n, s' = v + w1 (= (1-z)*n + z*s), Pool-copy
        s' to the output stage, and prehoist step t+1's x-dependent
        matmuls (tb broadcast row with start=True, Wx@x) into a fresh
        PSUM bank.

All gate scales/biases are folded into the doubled weights or added in
PSUM via broadcast-row matmuls, so every ACT op is plain func(x).  Each
chain-step owns one PSUM bank [128, 512] f32 (one start=True / one
stop=True per bank: start zeroes the whole 2KB zero-region).  Outputs
stream out as bf16 every CH steps; the host unpacks partitions to rows.
"""

import sys

import numpy as np

sys.path.insert(0, "/opt/trn_rl_repo")

import ml_dtypes  # noqa: E402

BF16 = ml_dtypes.bfloat16

T, B, C, D = 256, 64, 64, 64
NCORES = 8
ROWS = (B * C) // NCORES  # 512 rows per core
HALF = ROWS // 2  # 256 packed columns (2 rows per column)
CH = 8  # timesteps per DMA chunk
NPB = 3  # packed chains per core
PF = [86, 85, 85]  # packed columns per chain
POFF = [0, 86, 171, 256]

_PROGRAM_CACHE = {}


def _build_program():
    import concourse.bacc as bacc
    import concourse.tile as tile
    from concourse import mybir
    from contextlib import ExitStack

    BF = mybir.dt.bfloat16
    F32 = mybir.dt.float32
    AF = mybir.ActivationFunctionType
    OP = mybir.AluOpType

    nc = bacc.Bacc(None, target_bir_lowering=False, debug=False)

    xP = nc.dram_tensor("xP", [T, 2 * D, HALF], BF, kind="ExternalInput")
    s0 = nc.dram_tensor("s0", [2 * D, HALF], BF, kind="ExternalInput")
    tb = nc.dram_tensor("tb", [1, T, 2 * D], BF, kind="ExternalInput")
    # 8 block-diagonal doubled weight mats + identity, packed [128, 9*128]:
    #   wx, ws, wiz(-.5), whz(-.5), wir(+.5), whr(+.5), win, whn(.5), I
    wblob = nc.dram_tensor("wblob", [2 * D, 9 * 2 * D], BF, kind="ExternalInput")
    # bias rows on partition 0, side by side [1, 4*128]:
    # -0.5(b_iz+b_hz) | +0.5(b_ir+b_hr) | b_in | 0.5*b_hn  (each tiled 2x)
    bblob = nc.dram_tensor("bblob", [1, 4 * 2 * D], BF, kind="ExternalInput")
    outP = nc.dram_tensor("outP", [T, 2 * D, HALF], BF, kind="ExternalOutput")

    DD = 2 * D  # 128

    with ExitStack() as ctx:
        tc = ctx.enter_context(tile.TileContext(nc))
        consts = ctx.enter_context(tc.tile_pool(name="consts", bufs=1))
        xpool = ctx.enter_context(tc.tile_pool(name="xc", bufs=2))
        opool = ctx.enter_context(tc.tile_pool(name="ostage", bufs=2))
        spool = ctx.enter_context(tc.tile_pool(name="state", bufs=3))
        upool = ctx.enter_context(tc.tile_pool(name="u", bufs=2))
        gpool = ctx.enter_context(tc.tile_pool(name="gates", bufs=2))
        psum = ctx.enter_context(tc.tile_pool(name="psum", bufs=2, space="PSUM"))

        wblob_sb = consts.tile([DD, 9 * DD], BF, tag="wblob")
        nc.sync.dma_start(out=wblob_sb, in_=wblob[:, :])
        bblob_sb = consts.tile([1, 4 * DD], BF, tag="bblob")
        tb_sb = consts.tile([1, T, DD], BF, tag="tb")
        TBG = T // 8  # tb DMA granule: chunk 0 upfront, rest streamed

        def tb_dma(g):
            nc.sync.dma_start(
                out=tb_sb[:, g * TBG : (g + 1) * TBG, :],
                in_=tb[:, g * TBG : (g + 1) * TBG, :],
            )

        wx_sb = wblob_sb[:, 0 * DD : 1 * DD]
        ws_sb = wblob_sb[:, 1 * DD : 2 * DD]
        wiz_sb = wblob_sb[:, 2 * DD : 3 * DD]
        whz_sb = wblob_sb[:, 3 * DD : 4 * DD]
        wir_sb = wblob_sb[:, 4 * DD : 5 * DD]
        whr_sb = wblob_sb[:, 5 * DD : 6 * DD]
        win_sb = wblob_sb[:, 6 * DD : 7 * DD]
        whn_sb = wblob_sb[:, 7 * DD : 8 * DD]
        ident_sb = wblob_sb[:, 8 * DD : 9 * DD]
        bz_sb = bblob_sb[0:1, 0 * DD : 1 * DD]
        br_sb = bblob_sb[0:1, 1 * DD : 2 * DD]
        bin_sb = bblob_sb[0:1, 2 * DD : 3 * DD]
        bhn_sb = bblob_sb[0:1, 3 * DD : 4 * DD]
        ones_sb = consts.tile([1, HALF], BF)
        nc.vector.memset(ones_sb, 1.0)

        # --- pipeline state per chain ---
        R = []
        for p in range(NPB):
            st = spool.tile([DD, PF[p]], BF, tag=f"state{p}")
            nc.sync.dma_start(out=st, in_=s0[:, POFF[p] : POFF[p + 1]])
            R.append({"s": st})
        nc.sync.dma_start(out=bblob_sb, in_=bblob[:, :])
        tb_dma(0)

        chunks = {}  # chunk idx -> xc tile
        ostages = {}  # chunk idx -> ostage tile

        def get_chunk(c):
            if c not in chunks:
                t0 = c * CH
                xc = xpool.tile([DD, CH, HALF], BF, tag="xc", name="xc")
                nc.sync.dma_start(
                    out=xc, in_=xP[t0 : t0 + CH, :, :].rearrange("c p r -> p c r")
                )
                chunks[c] = xc
            return chunks[c]

        def get_ostage(c):
            if c not in ostages:
                ostages[c] = opool.tile([DD, CH, HALF], BF, tag="ostage", name="ostage")
            return ostages[c]

        def regions(bank, F):
            return (bank[:, 0:F], bank[:, F : 3 * F], bank[:, F : 2 * F],
                    bank[:, 2 * F : 3 * F], bank[:, 3 * F : 4 * F],
                    bank[:, 4 * F : 5 * F])

        def prehoist(p, t):
            """Fresh PSUM bank for (p, t): tb broadcast (start=True, zeroes
            the bank) + Wx@x.  Emitted one phase before A(t)."""
            F = PF[p]
            cs = slice(POFF[p], POFF[p + 1])
            bank = psum.tile([DD, 512], F32, tag=f"bank{p}", name=f"bank{p}")
            pm = bank[:, 0:F]
            nc.tensor.matmul(
                pm, tb_sb[:, t, :], ones_sb[:, 0:F], start=True, stop=False
            )
            xc = get_chunk(t // CH)
            nc.tensor.matmul(pm, wx_sb, xc[:, t % CH, cs], start=False, stop=False)
            R[p]["bank"] = bank

        def phaseA(p, t):
            F = PF[p]
            if p == 0 and t % TBG == CH and t + 3 * TBG // 4 < T:
                tb_dma(t // TBG + 1)
            s_b = R[p]["s"]
            R[p]["bank_cur"] = R[p]["bank"]
            bank = R[p]["bank_cur"]
            pm, zr, zz, rr, pw, hh = regions(bank, F)
            if t == 0:
                nc.tensor.matmul(pm, ws_sb, s_b, start=False, stop=False)
            nc.tensor.matmul(zz, bz_sb, ones_sb[:, 0:F], start=False, stop=False)
            nc.tensor.matmul(rr, br_sb, ones_sb[:, 0:F], start=False, stop=False)
            nc.tensor.matmul(zz, whz_sb, s_b, start=False, stop=False)
            nc.tensor.matmul(rr, whr_sb, s_b, start=False, stop=False)
            nc.tensor.matmul(hh, bhn_sb, ones_sb[:, 0:F], start=False, stop=False)
            nc.tensor.matmul(hh, whn_sb, s_b, start=False, stop=False)
            nc.tensor.matmul(pw, bin_sb, ones_sb[:, 0:F], start=False, stop=False)
            u = upool.tile([DD, F], BF, tag=f"u{p}", name=f"u{p}")
            nc.scalar.activation(u, pm, AF.Gelu)
            R[p]["u"] = u

        def phaseB(p, t):
            F = PF[p]
            s_b = R[p]["s"]
            bank = R[p]["bank_cur"]
            pm, zr, zz, rr, pw, hh = regions(bank, F)
            u = R[p]["u"]
            nc.tensor.matmul(zz, wiz_sb, u, start=False, stop=False)
            nc.tensor.matmul(rr, wir_sb, u, start=False, stop=False)
            nc.tensor.matmul(pw, win_sb, u, start=False, stop=False)
            hzr = gpool.tile([DD, 2 * F], BF, tag=f"hzr{p}", name=f"hzr{p}")
            nc.scalar.activation(hzr, zr, AF.Tanh)
            hz = hzr[:, 0:F]
            hr = hzr[:, F : 2 * F]
            # q = (hr + 1) * hh  (DVE; GPSIMD cannot access PSUM)
            q = gpool.tile([DD, F], BF, tag=f"q{p}", name=f"q{p}")
            nc.vector.scalar_tensor_tensor(q, hr, 1.0, hh, OP.add, OP.mult)
            nc.tensor.matmul(pw, ident_sb, q, start=False, stop=True)
            # blend coefficients: Q = 1-z, Q' = z;  w1 = z*s
            Q = gpool.tile([DD, F], BF, tag=f"Q{p}", name=f"Qc{p}")
            nc.vector.tensor_scalar(Q, hz, 0.5, 0.5, OP.mult, OP.add)
            Qp = gpool.tile([DD, F], BF, tag=f"Qp{p}", name=f"Qp{p}")
            nc.vector.tensor_scalar(Qp, hz, -0.5, 0.5, OP.mult, OP.add)
            w1 = gpool.tile([DD, F], BF, tag=f"w1{p}", name=f"w1{p}")
            nc.vector.tensor_mul(w1, Qp, s_b)
            R[p]["Q"] = Q
            R[p]["w1"] = w1
            # prehoist step t+1's x-side matmuls, then Ws@w1 (Ws@s' is
            # split linearly: Ws@s' = Ws@v + Ws@w1, so the gelu for t+1
            # never waits on s' itself)
            if t + 1 < T:
                prehoist(p, t + 1)
                nbank = R[p]["bank"]
                nc.tensor.matmul(
                    nbank[:, 0:F], ws_sb, w1, start=False, stop=False
                )

        def phaseC(p, t):
            F = PF[p]
            bank = R[p]["bank_cur"]
            pw = bank[:, 3 * F : 4 * F]
            n = gpool.tile([DD, F], BF, tag=f"n{p}", name=f"n{p}")
            nc.scalar.activation(n, pw, AF.Tanh)
            R[p]["n"] = n

        def phaseD(p, t):
            F = PF[p]
            cs = slice(POFF[p], POFF[p + 1])
            n = R[p]["n"]
            v = gpool.tile([DD, F], BF, tag=f"v{p}", name=f"v{p}")
            nc.vector.tensor_mul(v, R[p]["Q"], n)
            if t + 1 < T:
                nbank = R[p]["bank"]
                nc.tensor.matmul(nbank[:, 0:F], ws_sb, v, start=False, stop=False)
            s_nxt = spool.tile([DD, F], BF, tag=f"state{p}", name=f"state{p}")
            nc.vector.tensor_add(s_nxt, v, R[p]["w1"])
            ost = get_ostage(t // CH)
            nc.gpsimd.tensor_copy(out=ost[:, t % CH, cs], in_=s_nxt)
            R[p]["s"] = s_nxt
            if p == NPB - 1 and t % CH == CH - 1:
                c = t // CH
                nc.sync.dma_start(
                    out=outP[t - CH + 1 : t + 1, :, :].rearrange("c p r -> p c r"),
                    in_=ostages.pop(c),
                )
                chunks.pop(c, None)

        # --- bootstrap: bank(0) + x-mms for every chain ---
        for p in range(NPB):
            prehoist(p, 0)

        # --- pipelined slot loop: chain p does phase (i - p) % 4 ---
        PHASES = [phaseA, phaseB, phaseC, phaseD]
        for i in range(4 * T + 3):
            for p in range(NPB):
                ph = (i - p) % 4
                t = (i - p) // 4
                if 0 <= t < T:
                    PHASES[ph](p, t)

    nc.compile()
    return nc


def _blkdiag(a):
    """[64, 64] -> [128, 128] block-diagonal double."""
    out = np.zeros((2 * D, 2 * D), np.float32)
    out[:D, :D] = a
    out[D:, D:] = a
    return out


def _prep_host(x, mask, msg_W, msg_b, W_ih, W_hh, b_ih, b_hh, basis_freq, phase):
    """Host-side prep: partition-packing, sharding, weight doubling."""
    x = np.asarray(x, dtype=np.float32)
    mask = np.asarray(mask)
    msg_W = np.asarray(msg_W, np.float32)
    msg_b = np.asarray(msg_b, np.float32)
    W_ih = np.asarray(W_ih, np.float32)
    W_hh = np.asarray(W_hh, np.float32)
    b_ih = np.asarray(b_ih, np.float32)
    b_hh = np.asarray(b_hh, np.float32)
    basis_freq = np.asarray(basis_freq, np.float32)
    phase = np.asarray(phase, np.float32)

    tr = np.arange(T, dtype=np.int64) * mask.astype(np.int64)
    identity_gather = bool(np.array_equal(tr, np.arange(T)))

    xf = x.reshape(T, B * C, D)
    s0_rows = xf.mean(axis=0)  # [B*C, D] f32 (from ungathered x)
    if not identity_gather:
        xf = xf[tr]

    xP8, s08 = [], []
    for c in range(NCORES):
        blk = xf[:, c * ROWS : (c + 1) * ROWS, :]  # [T, 512, 64]
        lo = blk[:, 0:HALF].transpose(0, 2, 1)  # [T, 64, 256]
        hi = blk[:, HALF:ROWS].transpose(0, 2, 1)
        xP8.append(np.ascontiguousarray(
            np.concatenate([lo, hi], axis=1)).astype(BF16))
        sblk = s0_rows[c * ROWS : (c + 1) * ROWS]  # [512, 64]
        s08.append(np.ascontiguousarray(np.concatenate(
            [sblk[0:HALF].T, sblk[HALF:ROWS].T], axis=0)).astype(BF16))

    ts_ = np.arange(T, dtype=np.float32)[tr]
    te = np.cos(ts_[:, None] * basis_freq[None, :] + phase[None, :])  # [T, D]
    Wt = msg_W[:, 2 * D : 3 * D]
    tb1 = te @ Wt.T + msg_b[None, :]  # [T, 64]
    tb_host = np.tile(tb1, (1, 2)).astype(BF16).reshape(1, T, 2 * D)

    Wx = msg_W[:, 0:D].T
    Ws = msg_W[:, D : 2 * D].T
    # torch gate order in W_ih/W_hh: rows [r, z, n]
    Wir, Wiz, Win = W_ih[0:D], W_ih[D : 2 * D], W_ih[2 * D : 3 * D]
    Whr, Whz, Whn = W_hh[0:D], W_hh[D : 2 * D], W_hh[2 * D : 3 * D]

    mats = [
        Wx, Ws,
        -0.5 * Wiz.T, -0.5 * Whz.T,
        0.5 * Wir.T, 0.5 * Whr.T,
        Win.T, 0.5 * Whn.T,
        np.eye(D, dtype=np.float32),
    ]
    wblob = np.concatenate([_blkdiag(m) for m in mats], axis=1)

    bblob = np.concatenate([
        np.tile(-0.5 * (b_ih[D : 2 * D] + b_hh[D : 2 * D]), 2),
        np.tile(0.5 * (b_ih[0:D] + b_hh[0:D]), 2),
        np.tile(b_ih[2 * D : 3 * D], 2),
        np.tile(0.5 * b_hh[2 * D : 3 * D], 2),
    ]).reshape(1, 4 * 2 * D)

    shared = {
        "tb": tb_host,
        "wblob": wblob.astype(BF16),
        "bblob": bblob.astype(BF16),
    }
    in_maps = []
    for c in range(NCORES):
        m = dict(shared)
        m["xP"] = xP8[c]
        m["s0"] = s08[c]
        in_maps.append(m)
    return in_maps


def kernel(**inputs):
    from concourse.bass_utils import run_bass_kernel_spmd

    in_maps = _prep_host(**inputs)

    if "prog" not in _PROGRAM_CACHE:
        _PROGRAM_CACHE["prog"] = _build_program()
    nc = _PROGRAM_CACHE["prog"]

    res = run_bass_kernel_spmd(nc, in_maps, core_ids=list(range(NCORES)))
    _PROGRAM_CACHE["last_results"] = res

    out = np.empty((T, B * C, D), dtype=np.float32)
    for c in range(NCORES):
        outP_c = np.asarray(res.results[c]["outP"], dtype=np.float32)  # [T,128,256]
        base = c * ROWS
        out[:, base : base + HALF, :] = outP_c[:, 0:D, :].transpose(0, 2, 1)
        out[:, base + HALF : base + ROWS, :] = outP_c[:, D:, :].transpose(0, 2, 1)
    return out.reshape(T, B, C, D)
